# revision 30
# baseline (speedup 1.0000x reference)
"""Trainium2 Bass kernel for a GPT-style decoder block (S=2048, E=2048, H=16, D=128).

Sharding (per spec hint): tensor-parallel attention — core c owns heads
(2c, 2c+1) and computes Q/K/V and attention for those heads over the FULL
sequence; the attn c_proj is computed as a per-core partial over the
256-dim head slice and summed across cores with a single bf16
ReduceScatter, which also re-shards the residual stream sequence-parallel
(core c receives rows [256c, 256c+256)). LN2 + the FFN then run
sequence-parallel on the owned 256 rows with full FFN weights.

All GEMMs are bf16 with fp32 PSUM accumulation; the residual stream is
fp32 in SBUF. LayerNorm weights are folded into the following GEMM
weights on the host. The causal structure skips strictly-upper score and
attn-V blocks; diagonal blocks use a resident triangular mask applied to
the exp'd scores. exp uses a global -3 logit bias (cancels in softmax).
"""

import numpy as np
import ml_dtypes

import concourse.bass as bass
import concourse.mybir as mybir
import concourse.tile as tile
from concourse import bacc
from concourse.bass_utils import run_bass_kernel_spmd

P = 128
S, E, H, D = 2048, 2048, 16, 128
FH = 4 * E
NCORES = 8
NB = S // P          # 16 sequence blocks
EC = E // P          # 16 e-chunks
FC2 = FH // P        # 64 f-chunks
BF = mybir.dt.bfloat16
F8 = mybir.dt.float8e4
F32 = mybir.dt.float32
EPS = 1e-5
SCALE = 1.0 / np.sqrt(D)
EXP_BIAS = -3.0
AF = mybir.ActivationFunctionType
ALU = mybir.AluOpType

WFC_PRE = 0          # f2 chunks of wfc prefetched resident before the FFN
WPF_PRE = 4          # f2 chunks of wpf (np=0) prefetched resident


def _ln_stats(nc, small, x_sb, eps_tile, tag):
    """rowwise mean/var over E -> (mv, rstd, nmr = -mu*rstd)."""
    stats = small.tile([P, 4, 6], F32, name=f"st_{tag}", tag="st")
    for g in range(4):
        nc.vector.bn_stats(out=stats[:, g, :], in_=x_sb[:, g * 512:(g + 1) * 512])
    mv = small.tile([P, 2], F32, name=f"mv_{tag}", tag="mv")
    nc.vector.bn_aggr(out=mv[:], in_=stats[:])
    std = small.tile([P, 1], F32, name=f"sd_{tag}", tag="sd")
    nc.scalar.activation(out=std[:], in_=mv[:, 1:2], func=AF.Sqrt,
                         bias=eps_tile[:], scale=1.0)
    rstd = small.tile([P, 1], F32, name=f"rs_{tag}", tag="rs")
    nc.vector.reciprocal(out=rstd[:], in_=std[:])
    nmr = small.tile([P, 1], F32, name=f"nm_{tag}", tag="nm")
    nc.vector.tensor_scalar(out=nmr[:], in0=mv[:, 0:1], scalar1=rstd[:],
                            scalar2=-1.0, op0=ALU.mult, op1=ALU.mult)
    return mv, rstd, nmr


def build_program():
    nc = bacc.Bacc()

    xbf = nc.dram_tensor("xbf", [NB, P, E], BF, kind="ExternalInput")
    xf = nc.dram_tensor("xf", [2, P, E], F32, kind="ExternalInput")
    # wqk[p, rc, ec, r]: rc = (q_h0, q_h1, k_h0, k_h1)
    wqk = nc.dram_tensor("wqk", [P, 4, EC, P], BF, kind="ExternalInput")
    # wv[p, h, ec, d]
    wv = nc.dram_tensor("wv", [P, 2, EC, P], BF, kind="ExternalInput")
    # wpa[p, i, n, f]: c_proj slice (contraction hd = i*128+p)
    wpa = nc.dram_tensor("wpa", [P, 2, 4, 512], BF, kind="ExternalInput")
    # wfc[f2, p, ec, r]
    wfc = nc.dram_tensor("wfc", [FC2, P, EC, P], BF, kind="ExternalInput")
    # wpf[f2, p, n, e]
    wpf = nc.dram_tensor("wpf", [FC2, P, 4, 512], BF, kind="ExternalInput")
    ident_in = nc.dram_tensor("ident", [P, P], BF, kind="ExternalInput")
    tri_in = nc.dram_tensor("tri", [P, P], BF, kind="ExternalInput")
    out_own = nc.dram_tensor("out_own", [2, P, E], F32, kind="ExternalOutput")

    with tile.TileContext(nc) as tc:
        _body(nc, tc, xbf, xf, wqk, wv, wpa, wfc, wpf, ident_in, tri_in,
              out_own)
    nc.finalize()
    return nc


def _body(nc, tc, xbf, xf, wqk, wv, wpa, wfc, wpf, ident_in, tri_in, out_own):
    with tc.tile_pool(name="res", bufs=1) as res, \
         tc.tile_pool(name="small", bufs=4) as small, \
         tc.tile_pool(name="dram", bufs=1, space="DRAM") as dram:

        eps_tile = small.tile([P, 1], F32, name="eps_tile", tag="eps")
        nc.vector.memset(eps_tile[:], EPS)
        ebias_tile = res.tile([P, 1], F32, name="ebias_tile")
        nc.vector.memset(ebias_tile[:], EXP_BIAS)
        ident = res.tile([P, P], BF, name="ident_sb")
        nc.sync.dma_start(ident[:], ident_in[:])
        tri = res.tile([P, P], BF, name="tri_sb")
        nc.sync.dma_start(tri[:], tri_in[:])

        wpa_sb = res.tile([P, 2, 4, 512], BF, name="wpa_sb")
        xf_sb = []
        for j in range(2):
            x = res.tile([P, E], F32, name=f"xf_sb{j}")
            xf_sb.append(x)

        FSPLIT = 6
        attnres_cm = tc.tile_pool(name="attnres", bufs=1)
        attnres = attnres_cm.__enter__()
        qk_sb = attnres.tile([P, 4, S], BF, name="qk_sb")      # 16KB/part
        v_sb = attnres.tile([P, 2, NB, 132], BF, name="v_sb")  # 8.25KB/part
        attnT = attnres.tile([P, 2, S], BF, name="attnT_sb")   # 8KB/part
        nc.vector.memset(v_sb[:, :, :, 128:129], 1.0)

        # FFN weight prefetch (resident); DMAs issued interleaved below
        wpf_pre = res.tile([P, WPF_PRE, 2, 512], BF, name="wpf_pre")

        cc_in = dram.tile([S, E], BF, name="cc_in")
        cc_rs = [dram.tile([P, E], BF, name=f"cc_rs{k}") for k in range(2)]

        # ---------------- Phase A: LN1 -> hT, QKV, V ----------------
        with tc.tile_pool(name="pares", bufs=1) as pares, \
             tc.tile_pool(name="xstr", bufs=4) as xstr, \
             tc.tile_pool(name="h8p", bufs=2) as h8p, \
             tc.tile_pool(name="tps", bufs=2, space="PSUM") as tps, \
             tc.tile_pool(name="qkps", bufs=3, space="PSUM") as qkps, \
             tc.tile_pool(name="vps", bufs=2, space="PSUM") as vps:
            wqk_sb = pares.tile([P, 4, EC, P], BF, name="wqk_sb")
            wv_sb = pares.tile([P, 2, EC, P], BF, name="wv_sb")
            hT = pares.tile([P, EC, S], BF, name="hT_sb")    # 64KB/part
            for b in range(NB):
                x_sb = xstr.tile([P, E], BF, name="x_sb", tag="x")
                if b < 2:
                    for q in range(4):
                        qsl = slice(q * 512, (q + 1) * 512)
                        nc.sync.dma_start(x_sb[:, qsl], xbf[b, :, qsl])
                else:
                    nc.sync.dma_start(x_sb[:], xbf[b])
                if b == 1:
                    nc.sync.dma_start(wqk_sb[:], wqk[:])
                elif b == 4:
                    nc.sync.dma_start(wv_sb[:], wv[:])
                elif b == 5:
                    nc.sync.dma_start(wpa_sb[:], wpa[:])
                elif b == 6:
                    nc.sync.dma_start(xf_sb[0][:], xf[0])
                elif b == 7:
                    nc.sync.dma_start(xf_sb[1][:], xf[1])
                mv, rstd, nmr = _ln_stats(nc, small, x_sb, eps_tile, f"ln1_{b}")
                h8 = h8p.tile([P, E], BF, name="h8", tag="h8")
                if b % 2 == 0:
                    nc.scalar.activation(out=h8[:], in_=x_sb[:],
                                         func=AF.Identity, scale=rstd[:],
                                         bias=nmr[:])
                else:
                    nc.vector.tensor_scalar(out=h8[:], in0=x_sb[:],
                                            scalar1=mv[:, 0:1], scalar2=rstd[:],
                                            op0=ALU.subtract, op1=ALU.mult)
                for half in range(2):
                    tp = tps.tile([P, 8 * P], BF, name="tp", tag="tp")
                    for k in range(8):
                        e = half * 8 + k
                        nc.tensor.transpose(tp[:, k * P:(k + 1) * P],
                                            h8[:, e * P:(e + 1) * P], ident[:])
                    dst = hT[:, half * 8:(half + 1) * 8, b * P:(b + 1) * P]
                    if half == 0:
                        nc.vector.tensor_copy(dst, tp[:].rearrange(
                            "p (k s) -> p k s", k=8))
                    else:
                        nc.scalar.activation(
                            out=dst, in_=tp[:].rearrange("p (k s) -> p k s", k=8),
                            func=AF.Copy)

                if b % 4 == 3:
                    g = b // 4
                    sl = slice(g * 512, (g + 1) * 512)
                    for rc in range(4):
                        ps = qkps.tile([P, 512], F32, name="qk_ps", tag="qk")
                        for ec in range(EC):
                            nc.tensor.matmul(
                                ps[:], wqk_sb[:, rc, ec], hT[:, ec, sl],
                                start=(ec == 0), stop=(ec == EC - 1))
                        if rc % 2 == 0:
                            nc.scalar.activation(out=qk_sb[:, rc, sl],
                                                 in_=ps[:], func=AF.Copy)
                        else:
                            nc.vector.tensor_copy(qk_sb[:, rc, sl], ps[:])
                    for hh in range(2):
                        for tb in range(4 * g, 4 * g + 4):
                            psv = vps.tile([P, P], F32, name="v_ps", tag="v")
                            tsl = slice(tb * P, (tb + 1) * P)
                            for ec in range(EC):
                                nc.tensor.matmul(
                                    psv[:], hT[:, ec, tsl], wv_sb[:, hh, ec],
                                    start=(ec == 0), stop=(ec == EC - 1))
                            nc.vector.tensor_copy(v_sb[:, hh, tb, 0:P], psv[:])

        # ---------------- Phase B: attention (2 heads) ----------------
        wfck_cm = tc.tile_pool(name="wfck", bufs=1)
        wfck = wfck_cm.__enter__()
        kept = {}
        for f2 in range(FSPLIT):
            kept[f2] = wfck.tile([P, EC, P], BF, name=f"wfck{f2}")
        pre_dmas = [("kc", i) for i in range(FSPLIT)] + \
                   [("pf", i) for i in range(WPF_PRE)]
        pre_i = 0

        def issue_prefetch(k):
            nonlocal pre_i
            for _ in range(k):
                if pre_i >= len(pre_dmas):
                    return
                kind, i = pre_dmas[pre_i]
                pre_i += 1
                if kind == "kc":
                    nc.sync.dma_start(kept[i][:], wfc[i])
                else:
                    nc.sync.dma_start(wpf_pre[:, i], wpf[i, :, 0:2, :])

        with tc.tile_pool(name="expp", bufs=1) as expp, \
             tc.tile_pool(name="asm", bufs=4) as asm, \
             tc.tile_pool(name="ccst", bufs=3) as ccst, \
             tc.tile_pool(name="pB", bufs=2, space="PSUM") as pB:

            def scores_head(hh, expT):
                for tb in range(NB):
                    g0 = tb // 4
                    for g in range(g0, 4):
                        off = (tb % 4) * P if g == g0 else 0
                        ps = pB.tile([P, 512], F32, name="sc_ps", tag="sc")
                        nc.tensor.matmul(
                            ps[:, off:], qk_sb[:, 2 + hh, tb * P:(tb + 1) * P],
                            qk_sb[:, hh, g * 512 + off:(g + 1) * 512],
                            start=True, stop=True)
                        nc.scalar.activation(
                            out=expT[:, tb, g * 512 + off:(g + 1) * 512],
                            in_=ps[:, off:], func=AF.Exp, scale=float(SCALE),
                            bias=ebias_tile[:])
                    nc.gpsimd.tensor_mul(
                        expT[:, tb, tb * P:(tb + 1) * P],
                        expT[:, tb, tb * P:(tb + 1) * P], tri[:])
                    issue_prefetch(1)

            def attnv_sc(hh, expT, sc, tp):
                ssl = slice(sc * P, (sc + 1) * P)
                ps_at = pB.tile([P, 132], F32, name="at_ps", tag="at")
                for tb in range(sc + 1):
                    nc.tensor.matmul(
                        ps_at[:, 0:129], expT[:, tb, ssl],
                        v_sb[:, hh, tb, 0:129],
                        start=(tb == 0), stop=(tb == sc))
                recip = asm.tile([P, 1], F32, name="recip", tag="rc")
                nc.vector.reciprocal(recip[:], ps_at[:, 128:129])
                a8 = asm.tile([P, P], BF, name="a8", tag="a8")
                nc.vector.tensor_scalar_mul(a8[:], ps_at[:, 0:P], recip[:])
                nc.tensor.transpose(tp[:, (sc % 8) * P:(sc % 8 + 1) * P],
                                    a8[:], ident[:])

            def cproj_sc(sc):
                ssl = slice(sc * P, (sc + 1) * P)
                cp = ccst.tile([P, E], BF, name="cc_sb", tag="cc")
                for n in range(4):
                    psn = pB.tile([P, 512], F32, name="cp_ps", tag="cp")
                    for i in range(2):
                        nc.tensor.matmul(psn[:], attnT[:, i, ssl],
                                         wpa_sb[:, i, n, :],
                                         start=(i == 0), stop=(i == 1))
                    if n % 2 == 0:
                        nc.vector.tensor_copy(
                            cp[:, n * 512:(n + 1) * 512], psn[:])
                    else:
                        nc.scalar.activation(
                            out=cp[:, n * 512:(n + 1) * 512], in_=psn[:],
                            func=AF.Copy)
                nc.gpsimd.dma_start(cc_in[ssl], cp[:])

            def launch_rs(half):
                nc.gpsimd.collective_compute(
                    "ReduceScatter", ALU.add,
                    replica_groups=[list(range(NCORES))],
                    ins=[cc_in[half * 1024:(half + 1) * 1024].opt()],
                    outs=[cc_rs[half][:].opt()])

            expT0 = expp.tile([P, NB, S], BF, name="expT", tag="expT")
            scores_head(0, expT0)
            for sc0 in (0, 8):
                tp = pB.tile([P, 8 * P], BF, name="tr_ps", tag="tr")
                for sc in range(sc0, sc0 + 8):
                    attnv_sc(0, expT0, sc, tp)
                nc.vector.tensor_copy(
                    attnT[:, 0, sc0 * P:(sc0 + 8) * P], tp[:])
            expT1 = expp.tile([P, NB, S], BF, name="expT", tag="expT")
            scores_head(1, expT1)
            for sc0 in (0, 8):
                tp = pB.tile([P, 8 * P], BF, name="tr_ps", tag="tr")
                for sc in range(sc0, sc0 + 8):
                    attnv_sc(1, expT1, sc, tp)
                nc.vector.tensor_copy(
                    attnT[:, 1, sc0 * P:(sc0 + 8) * P], tp[:])
                for sc in range(sc0, sc0 + 8):
                    cproj_sc(sc)
                launch_rs(sc0 // 8)

        # ---------------- Phase D: z2 = x + r, LN2, h2T ----------------
        ffnres_cm = tc.tile_pool(name="ffnres", bufs=1)
        ffnres = ffnres_cm.__enter__()
        h2T = ffnres.tile([P, EC, 2 * P], BF, name="h2T_sb")
        with tc.tile_pool(name="rsb", bufs=2) as rsb, \
             tc.tile_pool(name="tps2", bufs=2, space="PSUM") as tps2:
            for j in range(2):
                with tc.tile_wait_until(1.0 + j):
                    r = rsb.tile([P, E], BF, name="r_sb", tag="r")
                    if j == 0:
                        nc.scalar.dma_start(r[:], cc_rs[j][:])
                    else:
                        nc.gpsimd.dma_start(r[:], cc_rs[j][:])
                    nc.vector.tensor_add(out=xf_sb[j][:], in0=xf_sb[j][:],
                                         in1=r[:])
                    mv2, rstd, nmr = _ln_stats(nc, small, xf_sb[j], eps_tile,
                                               f"ln2_{j}")
                    h28 = rsb.tile([P, E], BF, name="h28", tag="h28")
                    for half in range(2):
                        hsl = slice(half * 1024, (half + 1) * 1024)
                        nc.scalar.activation(out=h28[:, hsl],
                                             in_=xf_sb[j][:, hsl],
                                             func=AF.Identity, scale=rstd[:],
                                             bias=nmr[:])
                        tp = tps2.tile([P, 8 * P], BF, name="tp2", tag="tp2")
                        for k in range(8):
                            e = half * 8 + k
                            nc.tensor.transpose(tp[:, k * P:(k + 1) * P],
                                                h28[:, e * P:(e + 1) * P],
                                                ident[:])
                        nc.vector.tensor_copy(
                            h2T[:, half * 8:(half + 1) * 8, j * P:(j + 1) * P],
                            tp[:].rearrange("p (k s) -> p k s", k=8))

        # ---------------- Phase E: FFN ----------------
        gT = ffnres.tile([P, FC2, 2 * P], BF, name="gT_sb")
        with tc.tile_pool(name="wfcs", bufs=6) as wfcs, \
             tc.tile_pool(name="fcps", bufs=4, space="PSUM") as fcps, \
             tc.tile_pool(name="fcpsh", bufs=3, space="PSUM") as fcpsh:
            def kept_ap(f2, ec):
                return kept[f2][:, ec]

            # j0 halves first (overlaps RS#2 / j1 readback chain)
            for j in range(2):
                jsl = slice(j * P, (j + 1) * P)
                for f2 in range(FSPLIT):
                    ps = fcpsh.tile([P, P], F32, name="fch_ps", tag="fch")
                    for ec in range(EC):
                        nc.tensor.matmul(ps[:], kept_ap(f2, ec),
                                         h2T[:, ec, jsl],
                                         start=(ec == 0),
                                         stop=(ec == EC - 1))
                    nc.scalar.activation(out=gT[:, f2, jsl], in_=ps[:],
                                         func=AF.Gelu_apprx_tanh)
            for f2 in range(FSPLIT, FC2):
                wts = wfcs.tile([P, EC, P], BF, name="wfc_t", tag="wfc")
                nc.sync.dma_start(wts[:], wfc[f2])
                ps = fcps.tile([P, 2 * P], F32, name="fc_ps", tag="fc")
                for ec in range(EC):
                    nc.tensor.matmul(ps[:], wts[:, ec], h2T[:, ec, :],
                                     start=(ec == 0), stop=(ec == EC - 1))
                nc.scalar.activation(out=gT[:, f2, :], in_=ps[:],
                                     func=AF.Gelu_apprx_tanh)

        with tc.tile_pool(name="wpfs", bufs=6) as wpfs, \
             tc.tile_pool(name="pfps", bufs=1, space="PSUM") as pfps:
            for npair in range(2):
                ps2 = [[pfps.tile([P, 512], F32, name=f"pf{j}{ni}",
                                  tag=f"pf{j}{ni}") for ni in range(2)]
                       for j in range(2)]
                for f2 in range(FC2):
                    if npair == 0 and f2 < WPF_PRE:
                        def wt2_ap(ni, f2=f2):
                            return wpf_pre[:, f2, ni, :]
                    else:
                        wts = wpfs.tile([P, 2, 512], BF, name="wpf_t",
                                        tag="wpf")
                        nc.sync.dma_start(
                            wts[:], wpf[f2, :, 2 * npair:2 * npair + 2, :])

                        def wt2_ap(ni, wts=wts):
                            return wts[:, ni, :]
                    for j in range(2):
                        for ni in range(2):
                            nc.tensor.matmul(
                                ps2[j][ni][:],
                                gT[:, f2, j * P:(j + 1) * P],
                                wt2_ap(ni), start=(f2 == 0),
                                stop=(f2 == FC2 - 1))
                for j in range(2):
                    for ni in range(2):
                        n = 2 * npair + ni
                        nc.vector.tensor_add(
                            out=xf_sb[j][:, n * 512:(n + 1) * 512],
                            in0=xf_sb[j][:, n * 512:(n + 1) * 512],
                            in1=ps2[j][ni][:])
                        nc.sync.dma_start(
                            out_own[j, :, n * 512:(n + 1) * 512],
                            xf_sb[j][:, n * 512:(n + 1) * 512])
        ffnres_cm.__exit__(None, None, None)
        wfck_cm.__exit__(None, None, None)
        attnres_cm.__exit__(None, None, None)


# ------------------------------------------------------------------
# host side
# ------------------------------------------------------------------
_BF = ml_dtypes.bfloat16
_F8 = ml_dtypes.float8_e4m3


def _prep_shared(ln1_w, ln2_w, w_attn, w_fc, w_proj_ffn):
    w_attn = (w_attn * ln1_w[None, :]).astype(np.float32)
    w_fc = (w_fc * ln2_w[None, :]).astype(np.float32)
    # wfc[f2, p, ec, r]
    wfc = np.ascontiguousarray(
        w_fc.reshape(FC2, P, EC, P).transpose(0, 3, 2, 1)).astype(_BF)
    # wpf[f2, p, n, e]
    wpf = np.ascontiguousarray(
        w_proj_ffn.reshape(4, 512, FC2, P).transpose(2, 3, 0, 1)).astype(_BF)
    ident = np.eye(P, dtype=np.float32).astype(_BF)
    tri = (np.arange(P)[:, None] <= np.arange(P)[None, :]).astype(_BF)
    return w_attn, wfc, wpf, ident, tri


def _prep_core(c, w_attn_f, w_proj_attn):
    h0 = 2 * c
    rows = np.concatenate([
        w_attn_f[h0 * P:(h0 + 1) * P],
        w_attn_f[(h0 + 1) * P:(h0 + 2) * P],
        w_attn_f[E + h0 * P:E + (h0 + 1) * P],
        w_attn_f[E + (h0 + 1) * P:E + (h0 + 2) * P],
    ], axis=0).reshape(4, P, EC, P)
    wqk = np.ascontiguousarray(rows.transpose(3, 0, 2, 1)).astype(_BF)
    vrows = w_attn_f[2 * E + h0 * P:2 * E + (h0 + 2) * P].reshape(2, P, EC, P)
    wv = np.ascontiguousarray(vrows.transpose(3, 0, 2, 1)).astype(_BF)
    wpa = np.ascontiguousarray(
        w_proj_attn[:, c * 256:(c + 1) * 256].reshape(4, 512, 2, P)
        .transpose(3, 2, 0, 1)).astype(_BF)
    return wqk, wv, wpa


_CACHE = {}


def _get_program():
    if "nc" not in _CACHE:
        _CACHE["nc"] = build_program()
    return _CACHE["nc"]


def make_in_maps(x, ln1_w, ln2_w, w_attn, w_proj_attn, w_fc, w_proj_ffn):
    x = np.asarray(x, np.float32)
    w_attn_f, wfc, wpf, ident, tri = _prep_shared(
        np.asarray(ln1_w, np.float32), np.asarray(ln2_w, np.float32),
        np.asarray(w_attn, np.float32), np.asarray(w_fc, np.float32),
        np.asarray(w_proj_ffn, np.float32))
    w_proj_attn = np.asarray(w_proj_attn, np.float32)
    xbf = np.ascontiguousarray(x.reshape(NB, P, E)).astype(_BF)
    in_maps = []
    for c in range(NCORES):
        wqk, wv, wpa = _prep_core(c, w_attn_f, w_proj_attn)
        in_maps.append({
            "xbf": xbf,
            "xf": np.ascontiguousarray(x.reshape(NB, P, E)[[c, 8 + c]]),
            "wqk": wqk, "wv": wv, "wpa": wpa, "wfc": wfc, "wpf": wpf,
            "ident": ident, "tri": tri,
        })
    return in_maps


def kernel(x, ln1_w, ln2_w, w_attn, w_proj_attn, w_fc, w_proj_ffn):
    nc = _get_program()
    in_maps = make_in_maps(x, ln1_w, ln2_w, w_attn, w_proj_attn, w_fc,
                           w_proj_ffn)
    res = run_bass_kernel_spmd(nc, in_maps, core_ids=list(range(NCORES)))
    out = np.empty((S, E), np.float32)
    for c in range(NCORES):
        blk = res.results[c]["out_own"]
        out[c * P:(c + 1) * P] = blk[0]
        out[(8 + c) * P:(9 + c) * P] = blk[1]
    return out


if __name__ == "__main__":
    rng = np.random.default_rng(0)
    ins = {
        "x": rng.standard_normal((S, E), dtype=np.float32),
        "ln1_w": np.ones(E, np.float32),
        "ln2_w": np.ones(E, np.float32),
        "w_attn": (rng.standard_normal((3 * E, E), dtype=np.float32) * 0.02),
        "w_proj_attn": (rng.standard_normal((E, E), dtype=np.float32) * 0.02),
        "w_fc": (rng.standard_normal((FH, E), dtype=np.float32) * 0.02),
        "w_proj_ffn": (rng.standard_normal((E, FH), dtype=np.float32) * 0.02),
    }
    out = kernel(**ins)
    print("ran:", out.shape, out.dtype, np.abs(out).max())


# revision 31
# speedup vs baseline: 1.0070x; 1.0070x over previous
"""Trainium2 Bass kernel for a GPT-style decoder block (S=2048, E=2048, H=16, D=128).

Sharding (per spec hint): tensor-parallel attention — core c owns heads
(2c, 2c+1) and computes Q/K/V and attention for those heads over the FULL
sequence; the attn c_proj is computed as a per-core partial over the
256-dim head slice and summed across cores with a single bf16
ReduceScatter, which also re-shards the residual stream sequence-parallel
(core c receives rows [256c, 256c+256)). LN2 + the FFN then run
sequence-parallel on the owned 256 rows with full FFN weights.

All GEMMs are bf16 with fp32 PSUM accumulation; the residual stream is
fp32 in SBUF. LayerNorm weights are folded into the following GEMM
weights on the host. The causal structure skips strictly-upper score and
attn-V blocks; diagonal blocks use a resident triangular mask applied to
the exp'd scores. exp uses a global -3 logit bias (cancels in softmax).
"""

import numpy as np
import ml_dtypes

import concourse.bass as bass
import concourse.mybir as mybir
import concourse.tile as tile
from concourse import bacc
from concourse.bass_utils import run_bass_kernel_spmd

P = 128
S, E, H, D = 2048, 2048, 16, 128
FH = 4 * E
NCORES = 8
NB = S // P          # 16 sequence blocks
EC = E // P          # 16 e-chunks
FC2 = FH // P        # 64 f-chunks
BF = mybir.dt.bfloat16
F8 = mybir.dt.float8e4
F32 = mybir.dt.float32
EPS = 1e-5
SCALE = 1.0 / np.sqrt(D)
EXP_BIAS = -3.0
AF = mybir.ActivationFunctionType
ALU = mybir.AluOpType

WFC_PRE = 0          # f2 chunks of wfc prefetched resident before the FFN
WPF_PRE = 4          # f2 chunks of wpf (np=0) prefetched resident


def _ln_stats(nc, small, x_sb, eps_tile, tag):
    """rowwise mean/var over E -> (mv, rstd, nmr = -mu*rstd)."""
    stats = small.tile([P, 4, 6], F32, name=f"st_{tag}", tag="st")
    for g in range(4):
        nc.vector.bn_stats(out=stats[:, g, :], in_=x_sb[:, g * 512:(g + 1) * 512])
    mv = small.tile([P, 2], F32, name=f"mv_{tag}", tag="mv")
    nc.vector.bn_aggr(out=mv[:], in_=stats[:])
    std = small.tile([P, 1], F32, name=f"sd_{tag}", tag="sd")
    nc.scalar.activation(out=std[:], in_=mv[:, 1:2], func=AF.Sqrt,
                         bias=eps_tile[:], scale=1.0)
    rstd = small.tile([P, 1], F32, name=f"rs_{tag}", tag="rs")
    nc.vector.reciprocal(out=rstd[:], in_=std[:])
    nmr = small.tile([P, 1], F32, name=f"nm_{tag}", tag="nm")
    nc.vector.tensor_scalar(out=nmr[:], in0=mv[:, 0:1], scalar1=rstd[:],
                            scalar2=-1.0, op0=ALU.mult, op1=ALU.mult)
    return mv, rstd, nmr


def build_program():
    nc = bacc.Bacc()

    xbf = nc.dram_tensor("xbf", [NB, P, E], BF, kind="ExternalInput")
    xf = nc.dram_tensor("xf", [2, P, E], F32, kind="ExternalInput")
    # wqk[p, rc, ec, r]: rc = (q_h0, q_h1, k_h0, k_h1)
    wqk = nc.dram_tensor("wqk", [P, 4, EC, P], BF, kind="ExternalInput")
    # wv[p, h, ec, d]
    wv = nc.dram_tensor("wv", [P, 2, EC, P], BF, kind="ExternalInput")
    # wpa[p, i, n, f]: c_proj slice (contraction hd = i*128+p)
    wpa = nc.dram_tensor("wpa", [P, 2, 4, 512], BF, kind="ExternalInput")
    # wfc[f2, p, ec, r]
    wfc = nc.dram_tensor("wfc", [FC2, P, EC, P], BF, kind="ExternalInput")
    # wpf[f2, p, n, e]
    wpf = nc.dram_tensor("wpf", [FC2, P, 4, 512], BF, kind="ExternalInput")
    ident_in = nc.dram_tensor("ident", [P, P], BF, kind="ExternalInput")
    tri_in = nc.dram_tensor("tri", [P, P], BF, kind="ExternalInput")
    out_own = nc.dram_tensor("out_own", [2, P, E], F32, kind="ExternalOutput")

    with tile.TileContext(nc) as tc:
        _body(nc, tc, xbf, xf, wqk, wv, wpa, wfc, wpf, ident_in, tri_in,
              out_own)
    nc.finalize()
    return nc


def _body(nc, tc, xbf, xf, wqk, wv, wpa, wfc, wpf, ident_in, tri_in, out_own):
    with tc.tile_pool(name="res", bufs=1) as res, \
         tc.tile_pool(name="small", bufs=4) as small, \
         tc.tile_pool(name="dram", bufs=1, space="DRAM") as dram:

        eps_tile = small.tile([P, 1], F32, name="eps_tile", tag="eps")
        nc.vector.memset(eps_tile[:], EPS)
        ebias_tile = res.tile([P, 1], F32, name="ebias_tile")
        nc.vector.memset(ebias_tile[:], EXP_BIAS)
        ident = res.tile([P, P], BF, name="ident_sb")
        nc.sync.dma_start(ident[:], ident_in[:])
        tri = res.tile([P, P], BF, name="tri_sb")
        nc.sync.dma_start(tri[:], tri_in[:])

        wpa_sb = res.tile([P, 2, 4, 512], BF, name="wpa_sb")
        xf_sb = []
        for j in range(2):
            x = res.tile([P, E], F32, name=f"xf_sb{j}")
            xf_sb.append(x)

        FSPLIT = 6
        attnres_cm = tc.tile_pool(name="attnres", bufs=1)
        attnres = attnres_cm.__enter__()
        qk_sb = attnres.tile([P, 4, S], BF, name="qk_sb")      # 16KB/part
        v_sb = attnres.tile([P, 2, NB, 132], BF, name="v_sb")  # 8.25KB/part
        attnT = attnres.tile([P, 2, S], BF, name="attnT_sb")   # 8KB/part
        nc.vector.memset(v_sb[:, :, :, 128:129], 1.0)

        # FFN weight prefetch (resident); DMAs issued interleaved below
        wpf_pre = res.tile([P, WPF_PRE, 2, 512], BF, name="wpf_pre")

        cc_in = dram.tile([S, E], BF, name="cc_in")
        cc_rs = [dram.tile([P, E], BF, name=f"cc_rs{k}") for k in range(2)]

        # ---------------- Phase A: LN1 -> hT, QKV, V ----------------
        with tc.tile_pool(name="pares", bufs=1) as pares, \
             tc.tile_pool(name="xstr", bufs=4) as xstr, \
             tc.tile_pool(name="h8p", bufs=2) as h8p, \
             tc.tile_pool(name="tps", bufs=2, space="PSUM") as tps, \
             tc.tile_pool(name="qkps", bufs=3, space="PSUM") as qkps, \
             tc.tile_pool(name="vps", bufs=2, space="PSUM") as vps:
            wqk_sb = pares.tile([P, 4, EC, P], BF, name="wqk_sb")
            wv_sb = pares.tile([P, 2, EC, P], BF, name="wv_sb")
            hT = pares.tile([P, EC, S], BF, name="hT_sb")    # 64KB/part
            for b in range(NB):
                x_sb = xstr.tile([P, E], BF, name="x_sb", tag="x")
                if b < 2:
                    for q in range(4):
                        qsl = slice(q * 512, (q + 1) * 512)
                        nc.sync.dma_start(x_sb[:, qsl], xbf[b, :, qsl])
                else:
                    nc.sync.dma_start(x_sb[:], xbf[b])
                if b == 1:
                    nc.sync.dma_start(wqk_sb[:], wqk[:])
                elif b == 2:
                    nc.sync.dma_start(wv_sb[:], wv[:])
                elif b == 5:
                    nc.sync.dma_start(wpa_sb[:], wpa[:])
                elif b == 6:
                    nc.sync.dma_start(xf_sb[0][:], xf[0])
                elif b == 7:
                    nc.sync.dma_start(xf_sb[1][:], xf[1])
                mv, rstd, nmr = _ln_stats(nc, small, x_sb, eps_tile, f"ln1_{b}")
                h8 = h8p.tile([P, E], BF, name="h8", tag="h8")
                if b % 2 == 0:
                    nc.scalar.activation(out=h8[:], in_=x_sb[:],
                                         func=AF.Identity, scale=rstd[:],
                                         bias=nmr[:])
                else:
                    nc.vector.tensor_scalar(out=h8[:], in0=x_sb[:],
                                            scalar1=mv[:, 0:1], scalar2=rstd[:],
                                            op0=ALU.subtract, op1=ALU.mult)
                for half in range(2):
                    tp = tps.tile([P, 8 * P], BF, name="tp", tag="tp")
                    for k in range(8):
                        e = half * 8 + k
                        nc.tensor.transpose(tp[:, k * P:(k + 1) * P],
                                            h8[:, e * P:(e + 1) * P], ident[:])
                    dst = hT[:, half * 8:(half + 1) * 8, b * P:(b + 1) * P]
                    if half == 0:
                        nc.vector.tensor_copy(dst, tp[:].rearrange(
                            "p (k s) -> p k s", k=8))
                    else:
                        nc.scalar.activation(
                            out=dst, in_=tp[:].rearrange("p (k s) -> p k s", k=8),
                            func=AF.Copy)

                if b % 4 == 3:
                    g = b // 4
                    sl = slice(g * 512, (g + 1) * 512)
                    for rc in range(4):
                        ps = qkps.tile([P, 512], F32, name="qk_ps", tag="qk")
                        for ec in range(EC):
                            nc.tensor.matmul(
                                ps[:], wqk_sb[:, rc, ec], hT[:, ec, sl],
                                start=(ec == 0), stop=(ec == EC - 1))
                        if rc % 2 == 0:
                            nc.scalar.activation(out=qk_sb[:, rc, sl],
                                                 in_=ps[:], func=AF.Copy)
                        else:
                            nc.vector.tensor_copy(qk_sb[:, rc, sl], ps[:])
                    for hh in range(2):
                        for tb in range(4 * g, 4 * g + 4):
                            psv = vps.tile([P, P], F32, name="v_ps", tag="v")
                            tsl = slice(tb * P, (tb + 1) * P)
                            for ec in range(EC):
                                nc.tensor.matmul(
                                    psv[:], hT[:, ec, tsl], wv_sb[:, hh, ec],
                                    start=(ec == 0), stop=(ec == EC - 1))
                            nc.vector.tensor_copy(v_sb[:, hh, tb, 0:P], psv[:])

        # ---------------- Phase B: attention (2 heads) ----------------
        wfck_cm = tc.tile_pool(name="wfck", bufs=1)
        wfck = wfck_cm.__enter__()
        kept = {}
        for f2 in range(FSPLIT):
            kept[f2] = wfck.tile([P, EC, P], BF, name=f"wfck{f2}")
        pre_dmas = [("kc", i) for i in range(FSPLIT)] + \
                   [("pf", i) for i in range(WPF_PRE)]
        pre_i = 0

        def issue_prefetch(k):
            nonlocal pre_i
            for _ in range(k):
                if pre_i >= len(pre_dmas):
                    return
                kind, i = pre_dmas[pre_i]
                pre_i += 1
                if kind == "kc":
                    nc.sync.dma_start(kept[i][:], wfc[i])
                else:
                    nc.sync.dma_start(wpf_pre[:, i], wpf[i, :, 0:2, :])

        with tc.tile_pool(name="expp", bufs=1) as expp, \
             tc.tile_pool(name="asm", bufs=4) as asm, \
             tc.tile_pool(name="ccst", bufs=3) as ccst, \
             tc.tile_pool(name="pB", bufs=2, space="PSUM") as pB:

            def scores_head(hh, expT):
                for tb in range(NB):
                    g0 = tb // 4
                    for g in range(g0, 4):
                        off = (tb % 4) * P if g == g0 else 0
                        ps = pB.tile([P, 512], F32, name="sc_ps", tag="sc")
                        nc.tensor.matmul(
                            ps[:, off:], qk_sb[:, 2 + hh, tb * P:(tb + 1) * P],
                            qk_sb[:, hh, g * 512 + off:(g + 1) * 512],
                            start=True, stop=True)
                        nc.scalar.activation(
                            out=expT[:, tb, g * 512 + off:(g + 1) * 512],
                            in_=ps[:, off:], func=AF.Exp, scale=float(SCALE),
                            bias=ebias_tile[:])
                    nc.gpsimd.tensor_mul(
                        expT[:, tb, tb * P:(tb + 1) * P],
                        expT[:, tb, tb * P:(tb + 1) * P], tri[:])
                    issue_prefetch(1)

            def attnv_sc(hh, expT, sc, tp):
                ssl = slice(sc * P, (sc + 1) * P)
                ps_at = pB.tile([P, 132], F32, name="at_ps", tag="at")
                for tb in range(sc + 1):
                    nc.tensor.matmul(
                        ps_at[:, 0:129], expT[:, tb, ssl],
                        v_sb[:, hh, tb, 0:129],
                        start=(tb == 0), stop=(tb == sc))
                recip = asm.tile([P, 1], F32, name="recip", tag="rc")
                nc.vector.reciprocal(recip[:], ps_at[:, 128:129])
                a8 = asm.tile([P, P], BF, name="a8", tag="a8")
                nc.vector.tensor_scalar_mul(a8[:], ps_at[:, 0:P], recip[:])
                nc.tensor.transpose(tp[:, (sc % 8) * P:(sc % 8 + 1) * P],
                                    a8[:], ident[:])

            def cproj_sc(sc):
                ssl = slice(sc * P, (sc + 1) * P)
                cp = ccst.tile([P, E], BF, name="cc_sb", tag="cc")
                for n in range(4):
                    psn = pB.tile([P, 512], F32, name="cp_ps", tag="cp")
                    for i in range(2):
                        nc.tensor.matmul(psn[:], attnT[:, i, ssl],
                                         wpa_sb[:, i, n, :],
                                         start=(i == 0), stop=(i == 1))
                    if n % 2 == 0:
                        nc.vector.tensor_copy(
                            cp[:, n * 512:(n + 1) * 512], psn[:])
                    else:
                        nc.scalar.activation(
                            out=cp[:, n * 512:(n + 1) * 512], in_=psn[:],
                            func=AF.Copy)
                nc.gpsimd.dma_start(cc_in[ssl], cp[:])

            def launch_rs(half):
                nc.gpsimd.collective_compute(
                    "ReduceScatter", ALU.add,
                    replica_groups=[list(range(NCORES))],
                    ins=[cc_in[half * 1024:(half + 1) * 1024].opt()],
                    outs=[cc_rs[half][:].opt()])

            expT0 = expp.tile([P, NB, S], BF, name="expT", tag="expT")
            scores_head(0, expT0)
            for sc0 in (0, 8):
                tp = pB.tile([P, 8 * P], BF, name="tr_ps", tag="tr")
                for sc in range(sc0, sc0 + 8):
                    attnv_sc(0, expT0, sc, tp)
                nc.vector.tensor_copy(
                    attnT[:, 0, sc0 * P:(sc0 + 8) * P], tp[:])
            expT1 = expp.tile([P, NB, S], BF, name="expT", tag="expT")
            scores_head(1, expT1)
            for sc0 in (0, 8):
                tp = pB.tile([P, 8 * P], BF, name="tr_ps", tag="tr")
                for sc in range(sc0, sc0 + 8):
                    attnv_sc(1, expT1, sc, tp)
                nc.vector.tensor_copy(
                    attnT[:, 1, sc0 * P:(sc0 + 8) * P], tp[:])
                for sc in range(sc0, sc0 + 8):
                    cproj_sc(sc)
                launch_rs(sc0 // 8)

        # ---------------- Phase D: z2 = x + r, LN2, h2T ----------------
        ffnres_cm = tc.tile_pool(name="ffnres", bufs=1)
        ffnres = ffnres_cm.__enter__()
        h2T = ffnres.tile([P, EC, 2 * P], BF, name="h2T_sb")
        with tc.tile_pool(name="rsb", bufs=2) as rsb, \
             tc.tile_pool(name="tps2", bufs=2, space="PSUM") as tps2:
            for j in range(2):
                with tc.tile_wait_until(1.0 + j):
                    r = rsb.tile([P, E], BF, name="r_sb", tag="r")
                    if j == 0:
                        nc.scalar.dma_start(r[:], cc_rs[j][:])
                    else:
                        nc.gpsimd.dma_start(r[:], cc_rs[j][:])
                    nc.vector.tensor_add(out=xf_sb[j][:], in0=xf_sb[j][:],
                                         in1=r[:])
                    mv2, rstd, nmr = _ln_stats(nc, small, xf_sb[j], eps_tile,
                                               f"ln2_{j}")
                    h28 = rsb.tile([P, E], BF, name="h28", tag="h28")
                    for half in range(2):
                        hsl = slice(half * 1024, (half + 1) * 1024)
                        nc.scalar.activation(out=h28[:, hsl],
                                             in_=xf_sb[j][:, hsl],
                                             func=AF.Identity, scale=rstd[:],
                                             bias=nmr[:])
                        tp = tps2.tile([P, 8 * P], BF, name="tp2", tag="tp2")
                        for k in range(8):
                            e = half * 8 + k
                            nc.tensor.transpose(tp[:, k * P:(k + 1) * P],
                                                h28[:, e * P:(e + 1) * P],
                                                ident[:])
                        nc.vector.tensor_copy(
                            h2T[:, half * 8:(half + 1) * 8, j * P:(j + 1) * P],
                            tp[:].rearrange("p (k s) -> p k s", k=8))

        # ---------------- Phase E: FFN ----------------
        gT = ffnres.tile([P, FC2, 2 * P], BF, name="gT_sb")
        with tc.tile_pool(name="wfcs", bufs=6) as wfcs, \
             tc.tile_pool(name="fcps", bufs=4, space="PSUM") as fcps, \
             tc.tile_pool(name="fcpsh", bufs=3, space="PSUM") as fcpsh:
            def kept_ap(f2, ec):
                return kept[f2][:, ec]

            # j0 halves first (overlaps RS#2 / j1 readback chain)
            for j in range(2):
                jsl = slice(j * P, (j + 1) * P)
                for f2 in range(FSPLIT):
                    ps = fcpsh.tile([P, P], F32, name="fch_ps", tag="fch")
                    for ec in range(EC):
                        nc.tensor.matmul(ps[:], kept_ap(f2, ec),
                                         h2T[:, ec, jsl],
                                         start=(ec == 0),
                                         stop=(ec == EC - 1))
                    nc.scalar.activation(out=gT[:, f2, jsl], in_=ps[:],
                                         func=AF.Gelu_apprx_tanh)
            for f2 in range(FSPLIT, FC2):
                wts = wfcs.tile([P, EC, P], BF, name="wfc_t", tag="wfc")
                nc.sync.dma_start(wts[:], wfc[f2])
                ps = fcps.tile([P, 2 * P], F32, name="fc_ps", tag="fc")
                for ec in range(EC):
                    nc.tensor.matmul(ps[:], wts[:, ec], h2T[:, ec, :],
                                     start=(ec == 0), stop=(ec == EC - 1))
                nc.scalar.activation(out=gT[:, f2, :], in_=ps[:],
                                     func=AF.Gelu_apprx_tanh)

        with tc.tile_pool(name="wpfs", bufs=6) as wpfs, \
             tc.tile_pool(name="pfps", bufs=1, space="PSUM") as pfps:
            for npair in range(2):
                ps2 = [[pfps.tile([P, 512], F32, name=f"pf{j}{ni}",
                                  tag=f"pf{j}{ni}") for ni in range(2)]
                       for j in range(2)]
                for f2 in range(FC2):
                    if npair == 0 and f2 < WPF_PRE:
                        def wt2_ap(ni, f2=f2):
                            return wpf_pre[:, f2, ni, :]
                    else:
                        wts = wpfs.tile([P, 2, 512], BF, name="wpf_t",
                                        tag="wpf")
                        nc.sync.dma_start(
                            wts[:], wpf[f2, :, 2 * npair:2 * npair + 2, :])

                        def wt2_ap(ni, wts=wts):
                            return wts[:, ni, :]
                    for j in range(2):
                        for ni in range(2):
                            nc.tensor.matmul(
                                ps2[j][ni][:],
                                gT[:, f2, j * P:(j + 1) * P],
                                wt2_ap(ni), start=(f2 == 0),
                                stop=(f2 == FC2 - 1))
                for j in range(2):
                    for ni in range(2):
                        n = 2 * npair + ni
                        nc.vector.tensor_add(
                            out=xf_sb[j][:, n * 512:(n + 1) * 512],
                            in0=xf_sb[j][:, n * 512:(n + 1) * 512],
                            in1=ps2[j][ni][:])
                        nc.sync.dma_start(
                            out_own[j, :, n * 512:(n + 1) * 512],
                            xf_sb[j][:, n * 512:(n + 1) * 512])
        ffnres_cm.__exit__(None, None, None)
        wfck_cm.__exit__(None, None, None)
        attnres_cm.__exit__(None, None, None)


# ------------------------------------------------------------------
# host side
# ------------------------------------------------------------------
_BF = ml_dtypes.bfloat16
_F8 = ml_dtypes.float8_e4m3


def _prep_shared(ln1_w, ln2_w, w_attn, w_fc, w_proj_ffn):
    w_attn = (w_attn * ln1_w[None, :]).astype(np.float32)
    w_fc = (w_fc * ln2_w[None, :]).astype(np.float32)
    # wfc[f2, p, ec, r]
    wfc = np.ascontiguousarray(
        w_fc.reshape(FC2, P, EC, P).transpose(0, 3, 2, 1)).astype(_BF)
    # wpf[f2, p, n, e]
    wpf = np.ascontiguousarray(
        w_proj_ffn.reshape(4, 512, FC2, P).transpose(2, 3, 0, 1)).astype(_BF)
    ident = np.eye(P, dtype=np.float32).astype(_BF)
    tri = (np.arange(P)[:, None] <= np.arange(P)[None, :]).astype(_BF)
    return w_attn, wfc, wpf, ident, tri


def _prep_core(c, w_attn_f, w_proj_attn):
    h0 = 2 * c
    rows = np.concatenate([
        w_attn_f[h0 * P:(h0 + 1) * P],
        w_attn_f[(h0 + 1) * P:(h0 + 2) * P],
        w_attn_f[E + h0 * P:E + (h0 + 1) * P],
        w_attn_f[E + (h0 + 1) * P:E + (h0 + 2) * P],
    ], axis=0).reshape(4, P, EC, P)
    wqk = np.ascontiguousarray(rows.transpose(3, 0, 2, 1)).astype(_BF)
    vrows = w_attn_f[2 * E + h0 * P:2 * E + (h0 + 2) * P].reshape(2, P, EC, P)
    wv = np.ascontiguousarray(vrows.transpose(3, 0, 2, 1)).astype(_BF)
    wpa = np.ascontiguousarray(
        w_proj_attn[:, c * 256:(c + 1) * 256].reshape(4, 512, 2, P)
        .transpose(3, 2, 0, 1)).astype(_BF)
    return wqk, wv, wpa


_CACHE = {}


def _get_program():
    if "nc" not in _CACHE:
        _CACHE["nc"] = build_program()
    return _CACHE["nc"]


def make_in_maps(x, ln1_w, ln2_w, w_attn, w_proj_attn, w_fc, w_proj_ffn):
    x = np.asarray(x, np.float32)
    w_attn_f, wfc, wpf, ident, tri = _prep_shared(
        np.asarray(ln1_w, np.float32), np.asarray(ln2_w, np.float32),
        np.asarray(w_attn, np.float32), np.asarray(w_fc, np.float32),
        np.asarray(w_proj_ffn, np.float32))
    w_proj_attn = np.asarray(w_proj_attn, np.float32)
    xbf = np.ascontiguousarray(x.reshape(NB, P, E)).astype(_BF)
    in_maps = []
    for c in range(NCORES):
        wqk, wv, wpa = _prep_core(c, w_attn_f, w_proj_attn)
        in_maps.append({
            "xbf": xbf,
            "xf": np.ascontiguousarray(x.reshape(NB, P, E)[[c, 8 + c]]),
            "wqk": wqk, "wv": wv, "wpa": wpa, "wfc": wfc, "wpf": wpf,
            "ident": ident, "tri": tri,
        })
    return in_maps


def kernel(x, ln1_w, ln2_w, w_attn, w_proj_attn, w_fc, w_proj_ffn):
    nc = _get_program()
    in_maps = make_in_maps(x, ln1_w, ln2_w, w_attn, w_proj_attn, w_fc,
                           w_proj_ffn)
    res = run_bass_kernel_spmd(nc, in_maps, core_ids=list(range(NCORES)))
    out = np.empty((S, E), np.float32)
    for c in range(NCORES):
        blk = res.results[c]["out_own"]
        out[c * P:(c + 1) * P] = blk[0]
        out[(8 + c) * P:(9 + c) * P] = blk[1]
    return out


if __name__ == "__main__":
    rng = np.random.default_rng(0)
    ins = {
        "x": rng.standard_normal((S, E), dtype=np.float32),
        "ln1_w": np.ones(E, np.float32),
        "ln2_w": np.ones(E, np.float32),
        "w_attn": (rng.standard_normal((3 * E, E), dtype=np.float32) * 0.02),
        "w_proj_attn": (rng.standard_normal((E, E), dtype=np.float32) * 0.02),
        "w_fc": (rng.standard_normal((FH, E), dtype=np.float32) * 0.02),
        "w_proj_ffn": (rng.standard_normal((E, FH), dtype=np.float32) * 0.02),
    }
    out = kernel(**ins)
    print("ran:", out.shape, out.dtype, np.abs(out).max())


# revision 32
# speedup vs baseline: 1.0279x; 1.0207x over previous
"""Trainium2 Bass kernel for a GPT-style decoder block (S=2048, E=2048, H=16, D=128).

Sharding (per spec hint): tensor-parallel attention — core c owns heads
(2c, 2c+1) and computes Q/K/V and attention for those heads over the FULL
sequence; the attn c_proj is computed as a per-core partial over the
256-dim head slice and summed across cores with a single bf16
ReduceScatter, which also re-shards the residual stream sequence-parallel
(core c receives rows [256c, 256c+256)). LN2 + the FFN then run
sequence-parallel on the owned 256 rows with full FFN weights.

All GEMMs are bf16 with fp32 PSUM accumulation; the residual stream is
fp32 in SBUF. LayerNorm weights are folded into the following GEMM
weights on the host. The causal structure skips strictly-upper score and
attn-V blocks; diagonal blocks use a resident triangular mask applied to
the exp'd scores. exp uses a global -3 logit bias (cancels in softmax).
"""

import numpy as np
import ml_dtypes

import concourse.bass as bass
import concourse.mybir as mybir
import concourse.tile as tile
from concourse import bacc
from concourse.bass_utils import run_bass_kernel_spmd

P = 128
S, E, H, D = 2048, 2048, 16, 128
FH = 4 * E
NCORES = 8
NB = S // P          # 16 sequence blocks
EC = E // P          # 16 e-chunks
FC2 = FH // P        # 64 f-chunks
BF = mybir.dt.bfloat16
F8 = mybir.dt.float8e4
F32 = mybir.dt.float32
EPS = 1e-5
SCALE = 1.0 / np.sqrt(D)
EXP_BIAS = -3.0
AF = mybir.ActivationFunctionType
ALU = mybir.AluOpType

WFC_PRE = 0          # f2 chunks of wfc prefetched resident before the FFN
WPF_PRE = 4          # f2 chunks of wpf (np=0) prefetched resident


def _ln_stats(nc, small, x_sb, eps_tile, tag):
    """rowwise mean/var over E -> (mv, rstd, nmr = -mu*rstd)."""
    stats = small.tile([P, 4, 6], F32, name=f"st_{tag}", tag="st")
    for g in range(4):
        nc.vector.bn_stats(out=stats[:, g, :], in_=x_sb[:, g * 512:(g + 1) * 512])
    mv = small.tile([P, 2], F32, name=f"mv_{tag}", tag="mv")
    nc.vector.bn_aggr(out=mv[:], in_=stats[:])
    std = small.tile([P, 1], F32, name=f"sd_{tag}", tag="sd")
    nc.scalar.activation(out=std[:], in_=mv[:, 1:2], func=AF.Sqrt,
                         bias=eps_tile[:], scale=1.0)
    rstd = small.tile([P, 1], F32, name=f"rs_{tag}", tag="rs")
    nc.vector.reciprocal(out=rstd[:], in_=std[:])
    nmr = small.tile([P, 1], F32, name=f"nm_{tag}", tag="nm")
    nc.vector.tensor_scalar(out=nmr[:], in0=mv[:, 0:1], scalar1=rstd[:],
                            scalar2=-1.0, op0=ALU.mult, op1=ALU.mult)
    return mv, rstd, nmr


def build_program():
    nc = bacc.Bacc()

    xbf = nc.dram_tensor("xbf", [NB, P, E], BF, kind="ExternalInput")
    xf = nc.dram_tensor("xf", [2, P, E], F32, kind="ExternalInput")
    # wqk[p, rc, ec, r]: rc = (q_h0, q_h1, k_h0, k_h1)
    wqk = nc.dram_tensor("wqk", [P, 4, EC, P], BF, kind="ExternalInput")
    # wv[p, h, ec, d]
    wv = nc.dram_tensor("wv", [P, 2, EC, P], BF, kind="ExternalInput")
    # wpa[p, i, n, f]: c_proj slice (contraction hd = i*128+p)
    wpa = nc.dram_tensor("wpa", [P, 2, 4, 512], BF, kind="ExternalInput")
    # wfc[f2, p, ec, r]
    wfc = nc.dram_tensor("wfc", [FC2, P, EC, P], BF, kind="ExternalInput")
    # wpf[f2, p, n, e]
    wpf = nc.dram_tensor("wpf", [FC2, P, 4, 512], BF, kind="ExternalInput")
    ident_in = nc.dram_tensor("ident", [P, P], BF, kind="ExternalInput")
    tri_in = nc.dram_tensor("tri", [P, P], BF, kind="ExternalInput")
    out_own = nc.dram_tensor("out_own", [2, P, E], F32, kind="ExternalOutput")

    with tile.TileContext(nc) as tc:
        _body(nc, tc, xbf, xf, wqk, wv, wpa, wfc, wpf, ident_in, tri_in,
              out_own)
    nc.finalize()
    return nc


def _body(nc, tc, xbf, xf, wqk, wv, wpa, wfc, wpf, ident_in, tri_in, out_own):
    with tc.tile_pool(name="res", bufs=1) as res, \
         tc.tile_pool(name="small", bufs=4) as small, \
         tc.tile_pool(name="dram", bufs=1, space="DRAM") as dram:

        eps_tile = small.tile([P, 1], F32, name="eps_tile", tag="eps")
        nc.vector.memset(eps_tile[:], EPS)
        ebias_tile = res.tile([P, 1], F32, name="ebias_tile")
        nc.vector.memset(ebias_tile[:], EXP_BIAS)
        ident = res.tile([P, P], BF, name="ident_sb")
        nc.sync.dma_start(ident[:], ident_in[:])
        tri = res.tile([P, P], BF, name="tri_sb")
        nc.sync.dma_start(tri[:], tri_in[:])

        wpa_sb = res.tile([P, 2, 4, 512], BF, name="wpa_sb")
        xf_sb = []
        for j in range(2):
            x = res.tile([P, E], F32, name=f"xf_sb{j}")
            xf_sb.append(x)

        FSPLIT = 14
        attnres_cm = tc.tile_pool(name="attnres", bufs=1)
        attnres = attnres_cm.__enter__()
        qk_sb = attnres.tile([P, 4, S], BF, name="qk_sb")      # 16KB/part
        v_sb = attnres.tile([P, 2, NB, 132], BF, name="v_sb")  # 8.25KB/part
        attnT = attnres.tile([P, 2, S], BF, name="attnT_sb")   # 8KB/part
        nc.vector.memset(v_sb[:, :, :, 128:129], 1.0)

        # FFN weight prefetch (resident); DMAs issued interleaved below
        wpf_pre = res.tile([P, WPF_PRE, 2, 512], BF, name="wpf_pre")

        cc_in = dram.tile([S, E], BF, name="cc_in")
        cc_rs = [dram.tile([P, E], BF, name=f"cc_rs{k}") for k in range(2)]

        # ---------------- Phase A: LN1 -> hT, QKV, V ----------------
        with tc.tile_pool(name="pares", bufs=1) as pares, \
             tc.tile_pool(name="xstr", bufs=4) as xstr, \
             tc.tile_pool(name="h8p", bufs=2) as h8p, \
             tc.tile_pool(name="tps", bufs=2, space="PSUM") as tps, \
             tc.tile_pool(name="qkps", bufs=3, space="PSUM") as qkps, \
             tc.tile_pool(name="vps", bufs=2, space="PSUM") as vps:
            wqk_sb = pares.tile([P, 4, EC, P], BF, name="wqk_sb")
            wv_sb = pares.tile([P, 2, EC, P], BF, name="wv_sb")
            hT = pares.tile([P, EC, S], BF, name="hT_sb")    # 64KB/part
            for b in range(NB):
                x_sb = xstr.tile([P, E], BF, name="x_sb", tag="x")
                if b < 2:
                    for q in range(4):
                        qsl = slice(q * 512, (q + 1) * 512)
                        nc.sync.dma_start(x_sb[:, qsl], xbf[b, :, qsl])
                else:
                    nc.sync.dma_start(x_sb[:], xbf[b])
                if b == 1:
                    nc.sync.dma_start(wqk_sb[:], wqk[:])
                elif b == 2:
                    nc.sync.dma_start(wv_sb[:], wv[:])
                elif b == 5:
                    nc.sync.dma_start(wpa_sb[:], wpa[:])
                elif b == 6:
                    nc.sync.dma_start(xf_sb[0][:], xf[0])
                elif b == 7:
                    nc.sync.dma_start(xf_sb[1][:], xf[1])
                mv, rstd, nmr = _ln_stats(nc, small, x_sb, eps_tile, f"ln1_{b}")
                h8 = h8p.tile([P, E], BF, name="h8", tag="h8")
                if b % 2 == 0:
                    nc.scalar.activation(out=h8[:], in_=x_sb[:],
                                         func=AF.Identity, scale=rstd[:],
                                         bias=nmr[:])
                else:
                    nc.vector.tensor_scalar(out=h8[:], in0=x_sb[:],
                                            scalar1=mv[:, 0:1], scalar2=rstd[:],
                                            op0=ALU.subtract, op1=ALU.mult)
                for half in range(2):
                    tp = tps.tile([P, 8 * P], BF, name="tp", tag="tp")
                    for k in range(8):
                        e = half * 8 + k
                        nc.tensor.transpose(tp[:, k * P:(k + 1) * P],
                                            h8[:, e * P:(e + 1) * P], ident[:])
                    dst = hT[:, half * 8:(half + 1) * 8, b * P:(b + 1) * P]
                    if half == 0:
                        nc.vector.tensor_copy(dst, tp[:].rearrange(
                            "p (k s) -> p k s", k=8))
                    else:
                        nc.scalar.activation(
                            out=dst, in_=tp[:].rearrange("p (k s) -> p k s", k=8),
                            func=AF.Copy)

                if b % 4 == 3:
                    g = b // 4
                    sl = slice(g * 512, (g + 1) * 512)
                    for rc in range(4):
                        ps = qkps.tile([P, 512], F32, name="qk_ps", tag="qk")
                        for ec in range(EC):
                            nc.tensor.matmul(
                                ps[:], wqk_sb[:, rc, ec], hT[:, ec, sl],
                                start=(ec == 0), stop=(ec == EC - 1))
                        if rc % 2 == 0:
                            nc.scalar.activation(out=qk_sb[:, rc, sl],
                                                 in_=ps[:], func=AF.Copy)
                        else:
                            nc.vector.tensor_copy(qk_sb[:, rc, sl], ps[:])
                    for hh in range(2):
                        for tb in range(4 * g, 4 * g + 4):
                            psv = vps.tile([P, P], F32, name="v_ps", tag="v")
                            tsl = slice(tb * P, (tb + 1) * P)
                            for ec in range(EC):
                                nc.tensor.matmul(
                                    psv[:], hT[:, ec, tsl], wv_sb[:, hh, ec],
                                    start=(ec == 0), stop=(ec == EC - 1))
                            nc.vector.tensor_copy(v_sb[:, hh, tb, 0:P], psv[:])

        # ---------------- Phase B: attention (2 heads) ----------------
        wfck_cm = tc.tile_pool(name="wfck", bufs=1)
        wfck = wfck_cm.__enter__()
        kept = {}
        for f2 in range(FSPLIT):
            kept[f2] = wfck.tile([P, EC, P], BF, name=f"wfck{f2}")
        pre_dmas = [("kc", i) for i in range(FSPLIT)] + \
                   [("pf", i) for i in range(WPF_PRE)]
        pre_i = 0

        def issue_prefetch(k):
            nonlocal pre_i
            for _ in range(k):
                if pre_i >= len(pre_dmas):
                    return
                kind, i = pre_dmas[pre_i]
                pre_i += 1
                if kind == "kc":
                    nc.sync.dma_start(kept[i][:], wfc[i])
                else:
                    nc.sync.dma_start(wpf_pre[:, i], wpf[i, :, 0:2, :])

        with tc.tile_pool(name="expp", bufs=1) as expp, \
             tc.tile_pool(name="asm", bufs=4) as asm, \
             tc.tile_pool(name="ccst", bufs=3) as ccst, \
             tc.tile_pool(name="pB", bufs=2, space="PSUM") as pB:

            def scores_head(hh, elo, ehi):
                for tb in range(NB):
                    g0 = tb // 4
                    for g in range(g0, 4):
                        off = (tb % 4) * P if g == g0 else 0
                        ps = pB.tile([P, 512], F32, name="sc_ps", tag="sc")
                        nc.tensor.matmul(
                            ps[:, off:], qk_sb[:, 2 + hh, tb * P:(tb + 1) * P],
                            qk_sb[:, hh, g * 512 + off:(g + 1) * 512],
                            start=True, stop=True)
                        eT, col = (elo, g * 512) if g < 2 else \
                            (ehi, g * 512 - 1024)
                        nc.scalar.activation(
                            out=eT[:, tb, col + off:col + 512],
                            in_=ps[:, off:], func=AF.Exp, scale=float(SCALE),
                            bias=ebias_tile[:])
                    eT, col = (elo, tb * P) if tb < 8 else \
                        (ehi, tb * P - 1024)
                    nc.gpsimd.tensor_mul(
                        eT[:, tb, col:col + P],
                        eT[:, tb, col:col + P], tri[:])
                    issue_prefetch(1)

            def attnv_sc(hh, elo, ehi, sc, tp):
                eT = elo if sc < 8 else ehi
                ssl = slice(sc * P - (0 if sc < 8 else 1024),
                            (sc + 1) * P - (0 if sc < 8 else 1024))
                ps_at = pB.tile([P, 132], F32, name="at_ps", tag="at")
                for tb in range(sc + 1):
                    nc.tensor.matmul(
                        ps_at[:, 0:129], eT[:, tb, ssl],
                        v_sb[:, hh, tb, 0:129],
                        start=(tb == 0), stop=(tb == sc))
                recip = asm.tile([P, 1], F32, name="recip", tag="rc")
                nc.vector.reciprocal(recip[:], ps_at[:, 128:129])
                a8 = asm.tile([P, P], BF, name="a8", tag="a8")
                nc.vector.tensor_scalar_mul(a8[:], ps_at[:, 0:P], recip[:])
                nc.tensor.transpose(tp[:, (sc % 8) * P:(sc % 8 + 1) * P],
                                    a8[:], ident[:])

            def cproj_sc(sc):
                ssl = slice(sc * P, (sc + 1) * P)
                cp = ccst.tile([P, E], BF, name="cc_sb", tag="cc")
                for n in range(4):
                    psn = pB.tile([P, 512], F32, name="cp_ps", tag="cp")
                    for i in range(2):
                        nc.tensor.matmul(psn[:], attnT[:, i, ssl],
                                         wpa_sb[:, i, n, :],
                                         start=(i == 0), stop=(i == 1))
                    if n % 2 == 0:
                        nc.vector.tensor_copy(
                            cp[:, n * 512:(n + 1) * 512], psn[:])
                    else:
                        nc.scalar.activation(
                            out=cp[:, n * 512:(n + 1) * 512], in_=psn[:],
                            func=AF.Copy)
                nc.gpsimd.dma_start(cc_in[ssl], cp[:])

            def launch_rs(half):
                nc.gpsimd.collective_compute(
                    "ReduceScatter", ALU.add,
                    replica_groups=[list(range(NCORES))],
                    ins=[cc_in[half * 1024:(half + 1) * 1024].opt()],
                    outs=[cc_rs[half][:].opt()])

            elo0 = expp.tile([P, NB, 1024], BF, name="elo", tag="elo")
            ehi0 = expp.tile([P, NB, 1024], BF, name="ehi", tag="ehi")
            scores_head(0, elo0, ehi0)
            for sc0 in (0, 8):
                tp = pB.tile([P, 8 * P], BF, name="tr_ps", tag="tr")
                for sc in range(sc0, sc0 + 8):
                    attnv_sc(0, elo0, ehi0, sc, tp)
                nc.vector.tensor_copy(
                    attnT[:, 0, sc0 * P:(sc0 + 8) * P], tp[:])
            elo1 = expp.tile([P, NB, 1024], BF, name="elo", tag="elo")
            ehi1 = expp.tile([P, NB, 1024], BF, name="ehi", tag="ehi")
            scores_head(1, elo1, ehi1)
            for sc0 in (0, 8):
                tp = pB.tile([P, 8 * P], BF, name="tr_ps", tag="tr")
                for sc in range(sc0, sc0 + 8):
                    attnv_sc(1, elo1, ehi1, sc, tp)
                    nc.vector.tensor_copy(
                        attnT[:, 1, sc * P:(sc + 1) * P],
                        tp[:, (sc % 8) * P:(sc % 8 + 1) * P])
                    cproj_sc(sc)
                launch_rs(sc0 // 8)

        # ---------------- Phase D: z2 = x + r, LN2, h2T ----------------
        ffnres_cm = tc.tile_pool(name="ffnres", bufs=1)
        ffnres = ffnres_cm.__enter__()
        h2T = ffnres.tile([P, EC, 2 * P], BF, name="h2T_sb")
        with tc.tile_pool(name="rsb", bufs=2) as rsb, \
             tc.tile_pool(name="tps2", bufs=2, space="PSUM") as tps2:
            for j in range(2):
                with tc.tile_wait_until(1.0 + j):
                    r = rsb.tile([P, E], BF, name="r_sb", tag="r")
                    if j == 0:
                        nc.scalar.dma_start(r[:], cc_rs[j][:])
                    else:
                        nc.gpsimd.dma_start(r[:], cc_rs[j][:])
                    nc.vector.tensor_add(out=xf_sb[j][:], in0=xf_sb[j][:],
                                         in1=r[:])
                    mv2, rstd, nmr = _ln_stats(nc, small, xf_sb[j], eps_tile,
                                               f"ln2_{j}")
                    h28 = rsb.tile([P, E], BF, name="h28", tag="h28")
                    for half in range(2):
                        hsl = slice(half * 1024, (half + 1) * 1024)
                        nc.scalar.activation(out=h28[:, hsl],
                                             in_=xf_sb[j][:, hsl],
                                             func=AF.Identity, scale=rstd[:],
                                             bias=nmr[:])
                        tp = tps2.tile([P, 8 * P], BF, name="tp2", tag="tp2")
                        for k in range(8):
                            e = half * 8 + k
                            nc.tensor.transpose(tp[:, k * P:(k + 1) * P],
                                                h28[:, e * P:(e + 1) * P],
                                                ident[:])
                        nc.vector.tensor_copy(
                            h2T[:, half * 8:(half + 1) * 8, j * P:(j + 1) * P],
                            tp[:].rearrange("p (k s) -> p k s", k=8))

        # ---------------- Phase E: FFN ----------------
        gT = ffnres.tile([P, FC2, 2 * P], BF, name="gT_sb")
        with tc.tile_pool(name="wfcs", bufs=6) as wfcs, \
             tc.tile_pool(name="fcps", bufs=4, space="PSUM") as fcps, \
             tc.tile_pool(name="fcpsh", bufs=3, space="PSUM") as fcpsh:
            def kept_ap(f2, ec):
                return kept[f2][:, ec]

            # j0 halves first (overlaps RS#2 / j1 readback chain)
            for j in range(2):
                jsl = slice(j * P, (j + 1) * P)
                for f2 in range(FSPLIT):
                    ps = fcpsh.tile([P, P], F32, name="fch_ps", tag="fch")
                    for ec in range(EC):
                        nc.tensor.matmul(ps[:], kept_ap(f2, ec),
                                         h2T[:, ec, jsl],
                                         start=(ec == 0),
                                         stop=(ec == EC - 1))
                    nc.scalar.activation(out=gT[:, f2, jsl], in_=ps[:],
                                         func=AF.Gelu_apprx_tanh)
            for f2 in range(FSPLIT, FC2):
                wts = wfcs.tile([P, EC, P], BF, name="wfc_t", tag="wfc")
                nc.sync.dma_start(wts[:], wfc[f2])
                ps = fcps.tile([P, 2 * P], F32, name="fc_ps", tag="fc")
                for ec in range(EC):
                    nc.tensor.matmul(ps[:], wts[:, ec], h2T[:, ec, :],
                                     start=(ec == 0), stop=(ec == EC - 1))
                nc.scalar.activation(out=gT[:, f2, :], in_=ps[:],
                                     func=AF.Gelu_apprx_tanh)

        with tc.tile_pool(name="wpfs", bufs=6) as wpfs, \
             tc.tile_pool(name="pfps", bufs=1, space="PSUM") as pfps:
            for npair in range(2):
                ps2 = [[pfps.tile([P, 512], F32, name=f"pf{j}{ni}",
                                  tag=f"pf{j}{ni}") for ni in range(2)]
                       for j in range(2)]
                for f2 in range(FC2):
                    if npair == 0 and f2 < WPF_PRE:
                        def wt2_ap(ni, f2=f2):
                            return wpf_pre[:, f2, ni, :]
                    else:
                        wts = wpfs.tile([P, 2, 512], BF, name="wpf_t",
                                        tag="wpf")
                        nc.sync.dma_start(
                            wts[:], wpf[f2, :, 2 * npair:2 * npair + 2, :])

                        def wt2_ap(ni, wts=wts):
                            return wts[:, ni, :]
                    for j in range(2):
                        for ni in range(2):
                            nc.tensor.matmul(
                                ps2[j][ni][:],
                                gT[:, f2, j * P:(j + 1) * P],
                                wt2_ap(ni), start=(f2 == 0),
                                stop=(f2 == FC2 - 1))
                for j in range(2):
                    for ni in range(2):
                        n = 2 * npair + ni
                        nc.vector.tensor_add(
                            out=xf_sb[j][:, n * 512:(n + 1) * 512],
                            in0=xf_sb[j][:, n * 512:(n + 1) * 512],
                            in1=ps2[j][ni][:])
                        nc.sync.dma_start(
                            out_own[j, :, n * 512:(n + 1) * 512],
                            xf_sb[j][:, n * 512:(n + 1) * 512])
        ffnres_cm.__exit__(None, None, None)
        wfck_cm.__exit__(None, None, None)
        attnres_cm.__exit__(None, None, None)


# ------------------------------------------------------------------
# host side
# ------------------------------------------------------------------
_BF = ml_dtypes.bfloat16
_F8 = ml_dtypes.float8_e4m3


def _prep_shared(ln1_w, ln2_w, w_attn, w_fc, w_proj_ffn):
    w_attn = (w_attn * ln1_w[None, :]).astype(np.float32)
    w_fc = (w_fc * ln2_w[None, :]).astype(np.float32)
    # wfc[f2, p, ec, r]
    wfc = np.ascontiguousarray(
        w_fc.reshape(FC2, P, EC, P).transpose(0, 3, 2, 1)).astype(_BF)
    # wpf[f2, p, n, e]
    wpf = np.ascontiguousarray(
        w_proj_ffn.reshape(4, 512, FC2, P).transpose(2, 3, 0, 1)).astype(_BF)
    ident = np.eye(P, dtype=np.float32).astype(_BF)
    tri = (np.arange(P)[:, None] <= np.arange(P)[None, :]).astype(_BF)
    return w_attn, wfc, wpf, ident, tri


def _prep_core(c, w_attn_f, w_proj_attn):
    h0 = 2 * c
    rows = np.concatenate([
        w_attn_f[h0 * P:(h0 + 1) * P],
        w_attn_f[(h0 + 1) * P:(h0 + 2) * P],
        w_attn_f[E + h0 * P:E + (h0 + 1) * P],
        w_attn_f[E + (h0 + 1) * P:E + (h0 + 2) * P],
    ], axis=0).reshape(4, P, EC, P)
    wqk = np.ascontiguousarray(rows.transpose(3, 0, 2, 1)).astype(_BF)
    vrows = w_attn_f[2 * E + h0 * P:2 * E + (h0 + 2) * P].reshape(2, P, EC, P)
    wv = np.ascontiguousarray(vrows.transpose(3, 0, 2, 1)).astype(_BF)
    wpa = np.ascontiguousarray(
        w_proj_attn[:, c * 256:(c + 1) * 256].reshape(4, 512, 2, P)
        .transpose(3, 2, 0, 1)).astype(_BF)
    return wqk, wv, wpa


_CACHE = {}


def _get_program():
    if "nc" not in _CACHE:
        _CACHE["nc"] = build_program()
    return _CACHE["nc"]


def make_in_maps(x, ln1_w, ln2_w, w_attn, w_proj_attn, w_fc, w_proj_ffn):
    x = np.asarray(x, np.float32)
    w_attn_f, wfc, wpf, ident, tri = _prep_shared(
        np.asarray(ln1_w, np.float32), np.asarray(ln2_w, np.float32),
        np.asarray(w_attn, np.float32), np.asarray(w_fc, np.float32),
        np.asarray(w_proj_ffn, np.float32))
    w_proj_attn = np.asarray(w_proj_attn, np.float32)
    xbf = np.ascontiguousarray(x.reshape(NB, P, E)).astype(_BF)
    in_maps = []
    for c in range(NCORES):
        wqk, wv, wpa = _prep_core(c, w_attn_f, w_proj_attn)
        in_maps.append({
            "xbf": xbf,
            "xf": np.ascontiguousarray(x.reshape(NB, P, E)[[c, 8 + c]]),
            "wqk": wqk, "wv": wv, "wpa": wpa, "wfc": wfc, "wpf": wpf,
            "ident": ident, "tri": tri,
        })
    return in_maps


def kernel(x, ln1_w, ln2_w, w_attn, w_proj_attn, w_fc, w_proj_ffn):
    nc = _get_program()
    in_maps = make_in_maps(x, ln1_w, ln2_w, w_attn, w_proj_attn, w_fc,
                           w_proj_ffn)
    res = run_bass_kernel_spmd(nc, in_maps, core_ids=list(range(NCORES)))
    out = np.empty((S, E), np.float32)
    for c in range(NCORES):
        blk = res.results[c]["out_own"]
        out[c * P:(c + 1) * P] = blk[0]
        out[(8 + c) * P:(9 + c) * P] = blk[1]
    return out


if __name__ == "__main__":
    rng = np.random.default_rng(0)
    ins = {
        "x": rng.standard_normal((S, E), dtype=np.float32),
        "ln1_w": np.ones(E, np.float32),
        "ln2_w": np.ones(E, np.float32),
        "w_attn": (rng.standard_normal((3 * E, E), dtype=np.float32) * 0.02),
        "w_proj_attn": (rng.standard_normal((E, E), dtype=np.float32) * 0.02),
        "w_fc": (rng.standard_normal((FH, E), dtype=np.float32) * 0.02),
        "w_proj_ffn": (rng.standard_normal((E, FH), dtype=np.float32) * 0.02),
    }
    out = kernel(**ins)
    print("ran:", out.shape, out.dtype, np.abs(out).max())


# revision 36
# speedup vs baseline: 1.0310x; 1.0030x over previous
"""Trainium2 Bass kernel for a GPT-style decoder block (S=2048, E=2048, H=16, D=128).

Sharding (per spec hint): tensor-parallel attention — core c owns heads
(2c, 2c+1) and computes Q/K/V and attention for those heads over the FULL
sequence; the attn c_proj is computed as a per-core partial over the
256-dim head slice and summed across cores with a single bf16
ReduceScatter, which also re-shards the residual stream sequence-parallel
(core c receives rows [256c, 256c+256)). LN2 + the FFN then run
sequence-parallel on the owned 256 rows with full FFN weights.

All GEMMs are bf16 with fp32 PSUM accumulation; the residual stream is
fp32 in SBUF. LayerNorm weights are folded into the following GEMM
weights on the host. The causal structure skips strictly-upper score and
attn-V blocks; diagonal blocks use a resident triangular mask applied to
the exp'd scores. exp uses a global -3 logit bias (cancels in softmax).
"""

import numpy as np
import ml_dtypes

import concourse.bass as bass
import concourse.mybir as mybir
import concourse.tile as tile
from concourse import bacc
from concourse.bass_utils import run_bass_kernel_spmd

P = 128
S, E, H, D = 2048, 2048, 16, 128
FH = 4 * E
NCORES = 8
NB = S // P          # 16 sequence blocks
EC = E // P          # 16 e-chunks
FC2 = FH // P        # 64 f-chunks
BF = mybir.dt.bfloat16
F8 = mybir.dt.float8e4
F32 = mybir.dt.float32
EPS = 1e-5
SCALE = 1.0 / np.sqrt(D)
EXP_BIAS = -3.0
AF = mybir.ActivationFunctionType
ALU = mybir.AluOpType

WFC_PRE = 0          # f2 chunks of wfc prefetched resident before the FFN
WPF_PRE = 4          # f2 chunks of wpf (np=0) prefetched resident


def _ln_stats(nc, small, x_sb, eps_tile, tag):
    """rowwise mean/var over E -> (mv, rstd, nmr = -mu*rstd)."""
    stats = small.tile([P, 4, 6], F32, name=f"st_{tag}", tag="st")
    for g in range(4):
        nc.vector.bn_stats(out=stats[:, g, :], in_=x_sb[:, g * 512:(g + 1) * 512])
    mv = small.tile([P, 2], F32, name=f"mv_{tag}", tag="mv")
    nc.vector.bn_aggr(out=mv[:], in_=stats[:])
    std = small.tile([P, 1], F32, name=f"sd_{tag}", tag="sd")
    nc.scalar.activation(out=std[:], in_=mv[:, 1:2], func=AF.Sqrt,
                         bias=eps_tile[:], scale=1.0)
    rstd = small.tile([P, 1], F32, name=f"rs_{tag}", tag="rs")
    nc.vector.reciprocal(out=rstd[:], in_=std[:])
    nmr = small.tile([P, 1], F32, name=f"nm_{tag}", tag="nm")
    nc.vector.tensor_scalar(out=nmr[:], in0=mv[:, 0:1], scalar1=rstd[:],
                            scalar2=-1.0, op0=ALU.mult, op1=ALU.mult)
    return mv, rstd, nmr


def build_program():
    nc = bacc.Bacc()

    xbf = nc.dram_tensor("xbf", [NB, P, E], BF, kind="ExternalInput")
    xf = nc.dram_tensor("xf", [2, P, E], F32, kind="ExternalInput")
    # wqk[p, rc, ec, r]: rc = (q_h0, q_h1, k_h0, k_h1)
    wqk = nc.dram_tensor("wqk", [P, 4, EC, P], BF, kind="ExternalInput")
    # wv[p, h, ec, d]
    wv = nc.dram_tensor("wv", [P, 2, EC, P], BF, kind="ExternalInput")
    # wpa[p, i, n, f]: c_proj slice (contraction hd = i*128+p)
    wpa = nc.dram_tensor("wpa", [P, 2, 4, 512], BF, kind="ExternalInput")
    # wfc[f2, p, ec, r]
    wfc = nc.dram_tensor("wfc", [FC2, P, EC, P], BF, kind="ExternalInput")
    # wpf[f2, p, n, e]
    wpf = nc.dram_tensor("wpf", [FC2, P, 4, 512], BF, kind="ExternalInput")
    ident_in = nc.dram_tensor("ident", [P, P], BF, kind="ExternalInput")
    tri_in = nc.dram_tensor("tri", [P, P], F8, kind="ExternalInput")
    out_own = nc.dram_tensor("out_own", [2, P, E], F32, kind="ExternalOutput")

    with tile.TileContext(nc) as tc:
        _body(nc, tc, xbf, xf, wqk, wv, wpa, wfc, wpf, ident_in, tri_in,
              out_own)
    nc.finalize()
    return nc


def _body(nc, tc, xbf, xf, wqk, wv, wpa, wfc, wpf, ident_in, tri_in, out_own):
    with tc.tile_pool(name="res", bufs=1) as res, \
         tc.tile_pool(name="small", bufs=4) as small, \
         tc.tile_pool(name="dram", bufs=1, space="DRAM") as dram:

        eps_tile = small.tile([P, 1], F32, name="eps_tile", tag="eps")
        nc.vector.memset(eps_tile[:], EPS)
        ebias_tile = res.tile([P, 1], F32, name="ebias_tile")
        nc.vector.memset(ebias_tile[:], EXP_BIAS)
        ident = res.tile([P, P], BF, name="ident_sb")
        nc.sync.dma_start(ident[:], ident_in[:])
        tri = res.tile([P, P], F8, name="tri_sb")
        nc.sync.dma_start(tri[:], tri_in[:])

        wpa_sb = res.tile([P, 2, 4, 512], BF, name="wpa_sb")
        xf_sb = []
        for j in range(2):
            x = res.tile([P, E], F32, name=f"xf_sb{j}")
            xf_sb.append(x)

        FSPLIT = 14
        attnres_cm = tc.tile_pool(name="attnres", bufs=1)
        attnres = attnres_cm.__enter__()
        qk_sb = attnres.tile([P, 4, S], BF, name="qk_sb")      # 16KB/part
        v_sb = attnres.tile([P, 2, NB, 132], BF, name="v_sb")  # 8.25KB/part
        attnT = attnres.tile([P, 2, S], BF, name="attnT_sb")   # 8KB/part
        nc.vector.memset(v_sb[:, :, :, 128:129], 1.0)

        # FFN weight prefetch (resident); DMAs issued interleaved below
        wpf_pre = res.tile([P, WPF_PRE, 2, 512], BF, name="wpf_pre")

        cc_in = dram.tile([S, E], BF, name="cc_in")
        cc_rs = [dram.tile([P, E], BF, name=f"cc_rs{k}") for k in range(2)]

        # ---------------- Phase A: LN1 -> hT, QKV, V ----------------
        with tc.tile_pool(name="pares", bufs=1) as pares, \
             tc.tile_pool(name="xstr", bufs=4) as xstr, \
             tc.tile_pool(name="h8p", bufs=2) as h8p, \
             tc.tile_pool(name="tps", bufs=2, space="PSUM") as tps, \
             tc.tile_pool(name="qkps", bufs=3, space="PSUM") as qkps, \
             tc.tile_pool(name="vps", bufs=2, space="PSUM") as vps:
            wqk_sb = pares.tile([P, 4, EC, P], BF, name="wqk_sb")
            wv_sb = pares.tile([P, 2, EC, P], BF, name="wv_sb")
            hT = pares.tile([P, EC, S], BF, name="hT_sb")    # 64KB/part
            for b in range(NB):
                x_sb = xstr.tile([P, E], BF, name="x_sb", tag="x")
                if b < 2:
                    for q in range(4):
                        qsl = slice(q * 512, (q + 1) * 512)
                        nc.sync.dma_start(x_sb[:, qsl], xbf[b, :, qsl])
                else:
                    nc.sync.dma_start(x_sb[:], xbf[b])
                if b == 1:
                    nc.sync.dma_start(wqk_sb[:], wqk[:])
                elif b == 2:
                    nc.sync.dma_start(wv_sb[:], wv[:])
                elif b == 5:
                    nc.sync.dma_start(wpa_sb[:], wpa[:])
                elif b == 6:
                    nc.sync.dma_start(xf_sb[0][:], xf[0])
                elif b == 7:
                    nc.sync.dma_start(xf_sb[1][:], xf[1])
                mv, rstd, nmr = _ln_stats(nc, small, x_sb, eps_tile, f"ln1_{b}")
                h8 = h8p.tile([P, E], BF, name="h8", tag="h8")
                if b % 2 == 0:
                    nc.scalar.activation(out=h8[:], in_=x_sb[:],
                                         func=AF.Identity, scale=rstd[:],
                                         bias=nmr[:])
                else:
                    nc.vector.tensor_scalar(out=h8[:], in0=x_sb[:],
                                            scalar1=mv[:, 0:1], scalar2=rstd[:],
                                            op0=ALU.subtract, op1=ALU.mult)
                for half in range(2):
                    tp = tps.tile([P, 8 * P], BF, name="tp", tag="tp")
                    for k in range(8):
                        e = half * 8 + k
                        nc.tensor.transpose(tp[:, k * P:(k + 1) * P],
                                            h8[:, e * P:(e + 1) * P], ident[:])
                    dst = hT[:, half * 8:(half + 1) * 8, b * P:(b + 1) * P]
                    nc.vector.tensor_copy(dst, tp[:].rearrange(
                        "p (k s) -> p k s", k=8))

                if b % 4 == 3:
                    g = b // 4
                    sl = slice(g * 512, (g + 1) * 512)
                    for rc in range(4):
                        ps = qkps.tile([P, 512], F32, name="qk_ps", tag="qk")
                        for ec in range(EC):
                            nc.tensor.matmul(
                                ps[:], wqk_sb[:, rc, ec], hT[:, ec, sl],
                                start=(ec == 0), stop=(ec == EC - 1))
                        if rc % 2 == 0:
                            nc.scalar.activation(out=qk_sb[:, rc, sl],
                                                 in_=ps[:], func=AF.Copy)
                        else:
                            nc.vector.tensor_copy(qk_sb[:, rc, sl], ps[:])
                    for hh in range(2):
                        for tb in range(4 * g, 4 * g + 4):
                            psv = vps.tile([P, P], F32, name="v_ps", tag="v")
                            tsl = slice(tb * P, (tb + 1) * P)
                            for ec in range(EC):
                                nc.tensor.matmul(
                                    psv[:], hT[:, ec, tsl], wv_sb[:, hh, ec],
                                    start=(ec == 0), stop=(ec == EC - 1))
                            nc.vector.tensor_copy(v_sb[:, hh, tb, 0:P], psv[:])

        # ---------------- Phase B: attention (2 heads) ----------------
        wfck_cm = tc.tile_pool(name="wfck", bufs=1)
        wfck = wfck_cm.__enter__()
        kept = {}
        for f2 in range(FSPLIT):
            kept[f2] = wfck.tile([P, EC, P], BF, name=f"wfck{f2}")
        pre_dmas = [("kc", i) for i in range(FSPLIT)] + \
                   [("pf", i) for i in range(WPF_PRE)]
        pre_i = 0

        def issue_prefetch(k):
            nonlocal pre_i
            for _ in range(k):
                if pre_i >= len(pre_dmas):
                    return
                kind, i = pre_dmas[pre_i]
                pre_i += 1
                if kind == "kc":
                    nc.sync.dma_start(kept[i][:], wfc[i])
                else:
                    nc.sync.dma_start(wpf_pre[:, i], wpf[i, :, 0:2, :])

        with tc.tile_pool(name="expp", bufs=1) as expp, \
             tc.tile_pool(name="asm", bufs=4) as asm, \
             tc.tile_pool(name="ccst", bufs=3) as ccst, \
             tc.tile_pool(name="pB", bufs=2, space="PSUM") as pB:

            def scores_head(hh, elo, ehi):
                for tb in range(NB):
                    g0 = tb // 4
                    for g in range(g0, 4):
                        off = (tb % 4) * P if g == g0 else 0
                        ps = pB.tile([P, 512], F32, name="sc_ps", tag="sc")
                        nc.tensor.matmul(
                            ps[:, off:], qk_sb[:, 2 + hh, tb * P:(tb + 1) * P],
                            qk_sb[:, hh, g * 512 + off:(g + 1) * 512],
                            start=True, stop=True)
                        eT, col = (elo, g * 512) if g < 2 else \
                            (ehi, g * 512 - 1024)
                        nc.scalar.activation(
                            out=eT[:, tb, col + off:col + 512],
                            in_=ps[:, off:], func=AF.Exp, scale=float(SCALE),
                            bias=ebias_tile[:])
                    eT, col = (elo, tb * P) if tb < 8 else \
                        (ehi, tb * P - 1024)
                    nc.gpsimd.tensor_mul(
                        eT[:, tb, col:col + P],
                        eT[:, tb, col:col + P], tri[:])
                    issue_prefetch(1)

            def attnv_sc(hh, elo, ehi, sc, tp):
                eT = elo if sc < 8 else ehi
                ssl = slice(sc * P - (0 if sc < 8 else 1024),
                            (sc + 1) * P - (0 if sc < 8 else 1024))
                ps_at = pB.tile([P, 132], F32, name="at_ps", tag="at")
                for tb in range(sc + 1):
                    nc.tensor.matmul(
                        ps_at[:, 0:129], eT[:, tb, ssl],
                        v_sb[:, hh, tb, 0:129],
                        start=(tb == 0), stop=(tb == sc))
                recip = asm.tile([P, 1], F32, name="recip", tag="rc")
                nc.vector.reciprocal(recip[:], ps_at[:, 128:129])
                a8 = asm.tile([P, P], BF, name="a8", tag="a8")
                nc.vector.tensor_scalar_mul(a8[:], ps_at[:, 0:P], recip[:])
                nc.tensor.transpose(tp[:, (sc % 8) * P:(sc % 8 + 1) * P],
                                    a8[:], ident[:])

            def cproj_sc(sc):
                ssl = slice(sc * P, (sc + 1) * P)
                cp = ccst.tile([P, E], BF, name="cc_sb", tag="cc")
                for n in range(4):
                    psn = pB.tile([P, 512], F32, name="cp_ps", tag="cp")
                    for i in range(2):
                        nc.tensor.matmul(psn[:], attnT[:, i, ssl],
                                         wpa_sb[:, i, n, :],
                                         start=(i == 0), stop=(i == 1))
                    if n % 2 == 0:
                        nc.vector.tensor_copy(
                            cp[:, n * 512:(n + 1) * 512], psn[:])
                    else:
                        nc.scalar.activation(
                            out=cp[:, n * 512:(n + 1) * 512], in_=psn[:],
                            func=AF.Copy)
                nc.gpsimd.dma_start(cc_in[ssl], cp[:])

            def launch_rs(half):
                nc.gpsimd.collective_compute(
                    "ReduceScatter", ALU.add,
                    replica_groups=[list(range(NCORES))],
                    ins=[cc_in[half * 1024:(half + 1) * 1024].opt()],
                    outs=[cc_rs[half][:].opt()])

            elo0 = expp.tile([P, NB, 1024], F8, name="elo", tag="elo")
            ehi0 = expp.tile([P, NB, 1024], F8, name="ehi", tag="ehi")
            scores_head(0, elo0, ehi0)
            for sc0 in (0, 8):
                tp = pB.tile([P, 8 * P], BF, name="tr_ps", tag="tr")
                for sc in range(sc0, sc0 + 8):
                    attnv_sc(0, elo0, ehi0, sc, tp)
                nc.vector.tensor_copy(
                    attnT[:, 0, sc0 * P:(sc0 + 8) * P], tp[:])
            elo1 = expp.tile([P, NB, 1024], F8, name="elo", tag="elo")
            ehi1 = expp.tile([P, NB, 1024], F8, name="ehi", tag="ehi")
            scores_head(1, elo1, ehi1)
            for sc0 in (0, 8):
                tp = pB.tile([P, 8 * P], BF, name="tr_ps", tag="tr")
                for sc in range(sc0, sc0 + 8):
                    attnv_sc(1, elo1, ehi1, sc, tp)
                    nc.vector.tensor_copy(
                        attnT[:, 1, sc * P:(sc + 1) * P],
                        tp[:, (sc % 8) * P:(sc % 8 + 1) * P])
                    cproj_sc(sc)
                launch_rs(sc0 // 8)

        # ---------------- Phase D: z2 = x + r, LN2, h2T ----------------
        ffnres_cm = tc.tile_pool(name="ffnres", bufs=1)
        ffnres = ffnres_cm.__enter__()
        h2T = ffnres.tile([P, EC, 2 * P], BF, name="h2T_sb")
        with tc.tile_pool(name="rsb", bufs=2) as rsb, \
             tc.tile_pool(name="tps2", bufs=2, space="PSUM") as tps2:
            for j in range(2):
                with tc.tile_wait_until(1.0 + j):
                    r = rsb.tile([P, E], BF, name="r_sb", tag="r")
                    if j == 0:
                        nc.scalar.dma_start(r[:], cc_rs[j][:])
                    else:
                        nc.gpsimd.dma_start(r[:], cc_rs[j][:])
                    nc.vector.tensor_add(out=xf_sb[j][:], in0=xf_sb[j][:],
                                         in1=r[:])
                    mv2, rstd, nmr = _ln_stats(nc, small, xf_sb[j], eps_tile,
                                               f"ln2_{j}")
                    h28 = rsb.tile([P, E], BF, name="h28", tag="h28")
                    for half in range(2):
                        hsl = slice(half * 1024, (half + 1) * 1024)
                        nc.scalar.activation(out=h28[:, hsl],
                                             in_=xf_sb[j][:, hsl],
                                             func=AF.Identity, scale=rstd[:],
                                             bias=nmr[:])
                        tp = tps2.tile([P, 8 * P], BF, name="tp2", tag="tp2")
                        for k in range(8):
                            e = half * 8 + k
                            nc.tensor.transpose(tp[:, k * P:(k + 1) * P],
                                                h28[:, e * P:(e + 1) * P],
                                                ident[:])
                        nc.vector.tensor_copy(
                            h2T[:, half * 8:(half + 1) * 8, j * P:(j + 1) * P],
                            tp[:].rearrange("p (k s) -> p k s", k=8))

        # ---------------- Phase E: FFN ----------------
        gT = ffnres.tile([P, FC2, 2 * P], BF, name="gT_sb")
        with tc.tile_pool(name="wfcs", bufs=6) as wfcs, \
             tc.tile_pool(name="fcps", bufs=4, space="PSUM") as fcps, \
             tc.tile_pool(name="fcpsh", bufs=3, space="PSUM") as fcpsh:
            def kept_ap(f2, ec):
                return kept[f2][:, ec]

            # j0 halves first (overlaps RS#2 / j1 readback chain)
            for j in range(2):
                jsl = slice(j * P, (j + 1) * P)
                for f2 in range(FSPLIT):
                    ps = fcpsh.tile([P, P], F32, name="fch_ps", tag="fch")
                    for ec in range(EC):
                        nc.tensor.matmul(ps[:], kept_ap(f2, ec),
                                         h2T[:, ec, jsl],
                                         start=(ec == 0),
                                         stop=(ec == EC - 1))
                    nc.scalar.activation(out=gT[:, f2, jsl], in_=ps[:],
                                         func=AF.Gelu_apprx_tanh)
            for f2 in range(FSPLIT, FC2):
                wts = wfcs.tile([P, EC, P], BF, name="wfc_t", tag="wfc")
                nc.sync.dma_start(wts[:], wfc[f2])
                ps = fcps.tile([P, 2 * P], F32, name="fc_ps", tag="fc")
                for ec in range(EC):
                    nc.tensor.matmul(ps[:], wts[:, ec], h2T[:, ec, :],
                                     start=(ec == 0), stop=(ec == EC - 1))
                nc.scalar.activation(out=gT[:, f2, :], in_=ps[:],
                                     func=AF.Gelu_apprx_tanh)

        with tc.tile_pool(name="wpfs", bufs=6) as wpfs, \
             tc.tile_pool(name="pfps", bufs=1, space="PSUM") as pfps:
            for npair in range(2):
                ps2 = [[pfps.tile([P, 512], F32, name=f"pf{j}{ni}",
                                  tag=f"pf{j}{ni}") for ni in range(2)]
                       for j in range(2)]
                for f2 in range(FC2):
                    if npair == 0 and f2 < WPF_PRE:
                        def wt2_ap(ni, f2=f2):
                            return wpf_pre[:, f2, ni, :]
                    else:
                        wts = wpfs.tile([P, 2, 512], BF, name="wpf_t",
                                        tag="wpf")
                        nc.sync.dma_start(
                            wts[:], wpf[f2, :, 2 * npair:2 * npair + 2, :])

                        def wt2_ap(ni, wts=wts):
                            return wts[:, ni, :]
                    for j in range(2):
                        for ni in range(2):
                            nc.tensor.matmul(
                                ps2[j][ni][:],
                                gT[:, f2, j * P:(j + 1) * P],
                                wt2_ap(ni), start=(f2 == 0),
                                stop=(f2 == FC2 - 1))
                for j in range(2):
                    for ni in range(2):
                        n = 2 * npair + ni
                        nc.vector.tensor_add(
                            out=xf_sb[j][:, n * 512:(n + 1) * 512],
                            in0=xf_sb[j][:, n * 512:(n + 1) * 512],
                            in1=ps2[j][ni][:])
                        nc.sync.dma_start(
                            out_own[j, :, n * 512:(n + 1) * 512],
                            xf_sb[j][:, n * 512:(n + 1) * 512])
        ffnres_cm.__exit__(None, None, None)
        wfck_cm.__exit__(None, None, None)
        attnres_cm.__exit__(None, None, None)


# ------------------------------------------------------------------
# host side
# ------------------------------------------------------------------
_BF = ml_dtypes.bfloat16
_F8 = ml_dtypes.float8_e4m3


def _prep_shared(ln1_w, ln2_w, w_attn, w_fc, w_proj_ffn):
    w_attn = (w_attn * ln1_w[None, :]).astype(np.float32)
    w_fc = (w_fc * ln2_w[None, :]).astype(np.float32)
    # wfc[f2, p, ec, r]
    wfc = np.ascontiguousarray(
        w_fc.reshape(FC2, P, EC, P).transpose(0, 3, 2, 1)).astype(_BF)
    # wpf[f2, p, n, e]
    wpf = np.ascontiguousarray(
        w_proj_ffn.reshape(4, 512, FC2, P).transpose(2, 3, 0, 1)).astype(_BF)
    ident = np.eye(P, dtype=np.float32).astype(_BF)
    tri = (np.arange(P)[:, None] <= np.arange(P)[None, :]).astype(_F8)
    return w_attn, wfc, wpf, ident, tri


def _prep_core(c, w_attn_f, w_proj_attn):
    h0 = 2 * c
    rows = np.concatenate([
        w_attn_f[h0 * P:(h0 + 1) * P],
        w_attn_f[(h0 + 1) * P:(h0 + 2) * P],
        w_attn_f[E + h0 * P:E + (h0 + 1) * P],
        w_attn_f[E + (h0 + 1) * P:E + (h0 + 2) * P],
    ], axis=0).reshape(4, P, EC, P)
    wqk = np.ascontiguousarray(rows.transpose(3, 0, 2, 1)).astype(_BF)
    vrows = w_attn_f[2 * E + h0 * P:2 * E + (h0 + 2) * P].reshape(2, P, EC, P)
    wv = np.ascontiguousarray(vrows.transpose(3, 0, 2, 1)).astype(_BF)
    wpa = np.ascontiguousarray(
        w_proj_attn[:, c * 256:(c + 1) * 256].reshape(4, 512, 2, P)
        .transpose(3, 2, 0, 1)).astype(_BF)
    return wqk, wv, wpa


_CACHE = {}


def _get_program():
    if "nc" not in _CACHE:
        _CACHE["nc"] = build_program()
    return _CACHE["nc"]


def make_in_maps(x, ln1_w, ln2_w, w_attn, w_proj_attn, w_fc, w_proj_ffn):
    x = np.asarray(x, np.float32)
    w_attn_f, wfc, wpf, ident, tri = _prep_shared(
        np.asarray(ln1_w, np.float32), np.asarray(ln2_w, np.float32),
        np.asarray(w_attn, np.float32), np.asarray(w_fc, np.float32),
        np.asarray(w_proj_ffn, np.float32))
    w_proj_attn = np.asarray(w_proj_attn, np.float32)
    xbf = np.ascontiguousarray(x.reshape(NB, P, E)).astype(_BF)
    in_maps = []
    for c in range(NCORES):
        wqk, wv, wpa = _prep_core(c, w_attn_f, w_proj_attn)
        in_maps.append({
            "xbf": xbf,
            "xf": np.ascontiguousarray(x.reshape(NB, P, E)[[c, 8 + c]]),
            "wqk": wqk, "wv": wv, "wpa": wpa, "wfc": wfc, "wpf": wpf,
            "ident": ident, "tri": tri,
        })
    return in_maps


def kernel(x, ln1_w, ln2_w, w_attn, w_proj_attn, w_fc, w_proj_ffn):
    nc = _get_program()
    in_maps = make_in_maps(x, ln1_w, ln2_w, w_attn, w_proj_attn, w_fc,
                           w_proj_ffn)
    res = run_bass_kernel_spmd(nc, in_maps, core_ids=list(range(NCORES)))
    out = np.empty((S, E), np.float32)
    for c in range(NCORES):
        blk = res.results[c]["out_own"]
        out[c * P:(c + 1) * P] = blk[0]
        out[(8 + c) * P:(9 + c) * P] = blk[1]
    return out


if __name__ == "__main__":
    rng = np.random.default_rng(0)
    ins = {
        "x": rng.standard_normal((S, E), dtype=np.float32),
        "ln1_w": np.ones(E, np.float32),
        "ln2_w": np.ones(E, np.float32),
        "w_attn": (rng.standard_normal((3 * E, E), dtype=np.float32) * 0.02),
        "w_proj_attn": (rng.standard_normal((E, E), dtype=np.float32) * 0.02),
        "w_fc": (rng.standard_normal((FH, E), dtype=np.float32) * 0.02),
        "w_proj_ffn": (rng.standard_normal((E, FH), dtype=np.float32) * 0.02),
    }
    out = kernel(**ins)
    print("ran:", out.shape, out.dtype, np.abs(out).max())
